# revision 16
# baseline (speedup 1.0000x reference)
"""Multi-head attention (B=2, N=2048, D=1024, H=16) on 8 TRN2 NeuronCores.

Sharding: tensor-parallel over heads. Core c owns heads 2c, 2c+1 (a 128-wide
slice of the concat head dim). Each core:
  - projects Q^T, K^T, V^T (transposed layout [dh, rows]) for its heads over
    all B*N=4096 rows from host-transposed bf16 x^T inputs (W stationary,
    x moving -> all matmuls stream 512 moving cols)
  - V^T is PE-transposed (identity matmul) into natural [keys, dh] tiles with
    an appended ones-column per head (sumexp rides the PV matmul for free)
  - attention with transposed scores S^T[k, q] = K Q^T, exp on ScalarE
    (scale=1/8 folded in; no max-subtract needed: |scores/8| < ~4)
  - partial output projection out^T_c = Wo[:, slice] X_c^T -> [1024, 4096]
Host sums the 8 partial outputs and adds bo.

Schedule: one software-pipelined instruction stream. The attention kt loop is
scalar-bound (exp ~1.1us/kt vs ~0.86us of PE work), so all other PE work
(remaining projections, output projections) is queued as "filler" matmuls and
pumped between the scores and PV matmuls of each kt iteration; sentinels in
the filler queue let attention start as soon as the first row-tiles of K/V/Q
are projected (exp starts ~15us instead of ~50us). The PE stays continuously
busy, which also keeps the HAM clock-gate at full rate. PSUM: sg ring 2x2
banks + pv ring 2 banks + proj/out ring 2 banks = 8 banks exactly. The tail
(last q-tile) uses a PE-selector matmul instead of the slow SBUF-DMA shift
for the sumexp row, splits its output projection per head to skip the xT
assembly DMA, and runs junk warm-up matmuls through the normalize chain so
the final matmuls are not clock-gated.
"""

import sys

sys.path.insert(0, "/opt/trn_rl_repo")

from contextlib import ExitStack

import ml_dtypes
import numpy as np

import concourse.bass as bass
import concourse.mybir as mybir
import concourse.tile as tile
from concourse import bacc, masks
from concourse.bass_utils import run_bass_kernel_spmd

B, N, D, H, DH = 2, 2048, 1024, 16, 64
R = B * N  # 4096
NC = 8
HPC = H // NC  # 2 heads per core
DHC = HPC * DH  # 128 head dims per core
QT = 512  # query tile (psum bank of fp32)
KT = 128  # key tile (psum partitions)
NQT = N // QT  # 4
NKT = N // KT  # 16
KC = D // 128  # 8 contraction chunks

f32 = mybir.dt.float32
bf16 = mybir.dt.bfloat16

_cache = {}


def _fold(ap):
    # [D, X] dram -> [128, KC, X] partition-folded view for one-shot DMA
    return ap.rearrange("(a p) m -> p a m", p=128)


def _foldw(w):
    # [D, DHC] host weight -> [128, KC, DHC] partition-folded, contiguous
    return np.ascontiguousarray(w.reshape(KC, 128, DHC).transpose(1, 0, 2))


def build():
    if "nc" in _cache:
        return _cache["nc"]
    nc = bacc.Bacc("TRN2", target_bir_lowering=False, debug=False, num_devices=NC)
    xq = nc.dram_tensor("xqT", [D, R], bf16, kind="ExternalInput").ap()
    xk = nc.dram_tensor("xkT", [D, R], bf16, kind="ExternalInput").ap()
    xv = nc.dram_tensor("xvT", [D, R], bf16, kind="ExternalInput").ap()
    wq = nc.dram_tensor("wqT", [128, KC, DHC], bf16, kind="ExternalInput").ap()
    wk = nc.dram_tensor("wkT", [128, KC, DHC], bf16, kind="ExternalInput").ap()
    wv = nc.dram_tensor("wvT", [128, KC, DHC], bf16, kind="ExternalInput").ap()
    wo = nc.dram_tensor("woT", [DHC, D], bf16, kind="ExternalInput").ap()
    bq = nc.dram_tensor("bq", [DHC, 1], f32, kind="ExternalInput").ap()
    bk = nc.dram_tensor("bk", [DHC, 1], f32, kind="ExternalInput").ap()
    bv = nc.dram_tensor("bv", [DHC, 1], f32, kind="ExternalInput").ap()
    outT = nc.dram_tensor("outT", [D, R], bf16, kind="ExternalOutput").ap()

    with tile.TileContext(nc) as tc, ExitStack() as ctx:
        const = ctx.enter_context(tc.tile_pool(name="const", bufs=1))
        xpool = ctx.enter_context(tc.tile_pool(name="x", bufs=3))
        big = ctx.enter_context(tc.tile_pool(name="big", bufs=1))
        ppool = ctx.enter_context(tc.tile_pool(name="p", bufs=3))
        opool = ctx.enter_context(tc.tile_pool(name="o", bufs=4))
        npool = ctx.enter_context(tc.tile_pool(name="norm", bufs=2))
        ps_proj = ctx.enter_context(tc.tile_pool(name="psA", bufs=2, space="PSUM"))
        ps_s = ctx.enter_context(tc.tile_pool(name="psS", bufs=2, space="PSUM"))
        ps_pv = ctx.enter_context(tc.tile_pool(name="psPV", bufs=2, space="PSUM"))

        # ---- local SBUF constants / persistent tiles ----
        warm = const.tile([128, QT], bf16, tag="warm")
        nc.gpsimd.memset(warm[:], 0.0)
        ident = const.tile([128, 128], bf16, tag="ident")
        masks.make_identity(nc, ident[:])
        # f32 selector column: picks the sumexp row (partition 64) via matmul
        sel = const.tile([65, 1], f32, tag="sel")
        nc.vector.memset(sel[:], 0.0)
        nc.vector.memset(sel[64:65, :], 1.0)

        wk_sb = const.tile([128, KC, DHC], bf16, tag="wk")
        bk_sb = const.tile([DHC, 1], f32, tag="bk")
        wv_sb = const.tile([128, KC, DHC], bf16, tag="wv")
        bv_sb = const.tile([DHC, 1], f32, tag="bv")
        wq_sb = const.tile([128, KC, DHC], bf16, tag="wq")
        bq_sb = const.tile([DHC, 1], f32, tag="bq")
        wo_sb = const.tile([128, D], bf16, tag="wo")
        # partition-0 copy of Wo's h1 rows for the tail's per-head split
        woL_sb = const.tile([64, D], bf16, tag="woL")

        qTs, kTs, vTs, vs, xTs = [], [], [], [], []
        for b in range(B):
            qTs.append(big.tile([128, N], bf16, tag=f"qT{b}", name=f"qT{b}"))
            kTs.append(big.tile([128, N], bf16, tag=f"kT{b}", name=f"kT{b}"))
            vTs.append(big.tile([128, N], bf16, tag=f"vT{b}", name=f"vT{b}"))
            v = big.tile([128, NKT, HPC, 65], bf16, tag=f"v{b}", name=f"v{b}")
            nc.gpsimd.memset(v[:, :, :, 64:65], 1.0)
            vs.append(v)
            xTs.append(opool.tile([128, N], bf16, tag=f"xT{b}", name=f"xT{b}"))

        # ---- PE warm-up: garbage matmuls lift the HAM clock-gate while the
        # first DMAs land (outputs never read) ----
        def warm_mms(n, pool, tag):
            for _ in range(n):
                psw = pool.tile([65, QT], f32, tag=tag, name="psw")
                nc.tensor.matmul(
                    psw[:], warm[:, 0:65], warm[:], start=True, stop=True
                )

        warm_mms(8, ps_proj, "proj")

        # ---- preload the exp activation table while ScalarE is idle ----
        junk = const.tile([128, 1], bf16, tag="junk")
        nc.scalar.activation(
            junk[:], warm[:, 0:1], mybir.ActivationFunctionType.Exp, scale=0.125
        )

        # ================= filler machinery =================
        fill_q = []
        passed = set()

        def pump(ns):
            while fill_q and ns > 0:
                it = fill_q.pop(0)
                if isinstance(it, str):
                    passed.add(it)
                    continue
                ns -= it()

        def drain_to(tag):
            if tag in passed:
                return
            while fill_q:
                it = fill_q.pop(0)
                if isinstance(it, str):
                    passed.add(it)
                    if it == tag:
                        return
                    continue
                it()

        def flush():
            while fill_q:
                it = fill_q.pop(0)
                if isinstance(it, str):
                    passed.add(it)
                else:
                    it()

        def run_gen(g):
            for f in g:
                f()

        # ---- projection generator: dst[dh, rows] = W^T x^T (+ bias) ----
        # yields micro-ops; V transposes trail their chunk by one chunk so the
        # PE never waits on the bias-add.
        def gen_proj(b, dst, xdram, w_sb, b_sb, xw, t0, t1, split=False,
                     vtile=None):
            pend = []
            for xi in range(t0, t1):
                rlo = b * N + xi * xw
                xt = xpool.tile([128, KC, 1024], bf16, tag="x", name="xt")
                src = _fold(xdram[:, rlo : rlo + xw])
                if split:
                    def d1(xt=xt, src=src, xw=xw):
                        nc.sync.dma_start(xt[:, 0:1, 0:xw], src[:, 0:1, :])
                        return 0
                    yield d1
                    def d2(xt=xt, src=src, xw=xw):
                        nc.sync.dma_start(xt[:, 1:KC, 0:xw], src[:, 1:KC, :])
                        return 0
                    yield d2
                else:
                    def d0(xt=xt, src=src, xw=xw):
                        nc.sync.dma_start(xt[:, :, 0:xw], src)
                        return 0
                    yield d0
                for rl in range(xw // QT):
                    ps = ps_proj.tile([128, QT], f32, tag="proj", name="psp")
                    for kc in range(KC):
                        def mm(ps=ps, xt=xt, kc=kc, rl=rl):
                            nc.tensor.matmul(
                                ps[:],
                                w_sb[:, kc, :],
                                xt[:, kc, rl * QT : (rl + 1) * QT],
                                start=(kc == 0), stop=(kc == KC - 1),
                            )
                            return 215
                        yield mm
                    rt = (xi * xw + rl * QT) // QT
                    def badd(ps=ps, rt=rt):
                        nc.vector.tensor_scalar_add(
                            dst[:, rt * QT : (rt + 1) * QT], ps[:], b_sb[:]
                        )
                        return 0
                    yield badd
                    yield from pend
                    pend = []
                    if vtile is not None:
                        for kt in range(rt * 4, rt * 4 + 4):
                            pst = ps_proj.tile(
                                [128, 128], bf16, tag="proj", name="tp"
                            )
                            def tr(kt=kt, pst=pst):
                                nc.tensor.transpose(
                                    pst[:],
                                    dst[:, kt * 128 : (kt + 1) * 128],
                                    ident[:],
                                )
                                return 215
                            pend.append(tr)
                            def trc(kt=kt, pst=pst, vtile=vtile):
                                nc.vector.tensor_copy(
                                    vtile[:, kt, :, 0:64],
                                    pst[:].rearrange("p (h c) -> p h c", h=HPC),
                                )
                                return 0
                            pend.append(trc)
            yield from pend

        # ---- output projection generator for one (b, qt) ----
        def gen_outproj(b, qt):
            qs = slice(qt * QT, (qt + 1) * QT)
            for ot in range(KC):
                ps = ps_proj.tile([128, QT], f32, tag="proj", name="pso")
                def mm(ps=ps, ot=ot, b=b):
                    nc.tensor.matmul(
                        ps[:],
                        wo_sb[:, ot * 128 : (ot + 1) * 128],
                        xTs[b][:, qs],
                        start=True, stop=True,
                    )
                    return 215
                yield mm
                ob = opool.tile([128, QT], bf16, tag="o", name="ob")
                def cst(ps=ps, ob=ob):
                    nc.vector.tensor_copy(ob[:], ps[:])
                    return 0
                yield cst
                def st(ob=ob, ot=ot, b=b, qt=qt):
                    nc.gpsimd.dma_start(
                        outT[
                            ot * 128 : (ot + 1) * 128,
                            b * N + qt * QT : b * N + (qt + 1) * QT,
                        ],
                        ob[:],
                    )
                    return 0
                yield st

        # ---- attention for one (b, qt): software-pipelined kt loop ----
        def attention_qt(b, qt, budget, kneed=None, vneed=None):
            qs = slice(qt * QT, (qt + 1) * QT)
            pvs = [
                ps_pv.tile([65, QT], f32, tag="pv", name=f"pv{h}")
                for h in range(HPC)
            ]
            sgs, pts = {}, {}

            def scores(kt):
                if kneed and kt in kneed:
                    drain_to(kneed[kt])
                sg = ps_s.tile([128, 2 * QT], f32, tag="sg", name="sg")
                sgs[kt] = sg
                ks = slice(kt * KT, (kt + 1) * KT)
                for h in range(HPC):
                    hp = slice(64 * h, 64 * h + 64)
                    nc.tensor.matmul(
                        sg[:, h * QT : (h + 1) * QT],
                        kTs[b][hp, ks],
                        qTs[b][hp, qs],
                        start=True, stop=True,
                    )

            def expx(kt):
                pt = ppool.tile([128, 2 * QT], bf16, tag="pt", name="pt")
                pts[kt] = pt
                nc.scalar.activation(
                    pt[:], sgs.pop(kt)[:],
                    mybir.ActivationFunctionType.Exp, scale=0.125,
                )

            def pv(kt):
                if vneed and kt in vneed:
                    drain_to(vneed[kt])
                pt = pts.pop(kt)
                for h in range(HPC):
                    nc.tensor.matmul(
                        pvs[h][:],
                        vs[b][:, kt, h, :],
                        pt[:, h * QT : (h + 1) * QT],
                        start=(kt == 0), stop=(kt == NKT - 1),
                    )

            scores(0)
            expx(0)
            for kt in range(NKT):
                if kt + 1 < NKT:
                    scores(kt + 1)
                    expx(kt + 1)
                pump(budget)
                pv(kt)
            return pvs

        # ---- normalize X = PV / sumexp into xTs[b][:, qs] ----
        def normalize(b, qt, pvs):
            qs = slice(qt * QT, (qt + 1) * QT)
            for h in range(HPC):
                pvsb = npool.tile([65, QT], f32, tag="pvsb", name=f"pvsb{h}")
                nc.vector.tensor_copy(pvsb[:], pvs[h][:])
                rc = npool.tile([1, QT], f32, tag="rc", name=f"rc{h}")
                nc.sync.dma_start(rc[:], pvsb[64:65, :])
                nc.vector.reciprocal_approx_fast(rc[:], rc[:])
                rb = npool.tile([64, QT], f32, tag="rb", name=f"rb{h}")
                nc.gpsimd.partition_broadcast(rb[:], rc[:])
                if h == 0:
                    nc.vector.tensor_mul(xTs[b][0:64, qs], pvsb[0:64, :], rb[:])
                else:
                    tmp = npool.tile([64, QT], bf16, tag="tmp", name="tmp")
                    nc.vector.tensor_mul(tmp[:], pvsb[0:64, :], rb[:])
                    nc.sync.dma_start(xTs[b][64:128, qs], tmp[:])

        # ---- tail: last q-tile normalize + per-head-split output proj ----
        def tail_qt(b, qt, pvs):
            qs = slice(qt * QT, (qt + 1) * QT)
            xh, zr = [], []
            # copies first (h1 on ScalarE so the two heads run in parallel)
            pvsbs = []
            for h in range(HPC):
                pvsb = npool.tile([65, QT], f32, tag="pvsb", name=f"tpv{h}")
                if h == 0:
                    nc.vector.tensor_copy(pvsb[:], pvs[h][:])
                else:
                    nc.scalar.copy(pvsb[:], pvs[h][:])
                pvsbs.append(pvsb)
            # z-row shift via PE selector matmul (identity column 64)
            for h in range(HPC):
                zp = ps_pv.tile([1, QT], f32, tag="pv", name=f"zp{h}")
                nc.tensor.matmul(
                    zp[:], sel[:], pvsbs[h][:], start=True, stop=True
                )
                zr.append(zp)
            # junk matmuls keep the PE busy (HAM stays at full clock) while
            # the normalize chain runs on DVE/GpSimd
            warm_mms(12, ps_s, "sg")
            for h in range(HPC):
                rc = npool.tile([1, QT], f32, tag="rc", name=f"trc{h}")
                nc.vector.reciprocal_approx_fast(rc[:], zr[h][:])
                rb = npool.tile([64, QT], f32, tag="rb", name=f"trb{h}")
                nc.gpsimd.partition_broadcast(rb[:], rc[:])
                xt = npool.tile([64, QT], bf16, tag="tmp", name=f"tx{h}")
                if h == 0:
                    nc.vector.tensor_mul(xt[:], pvsbs[h][0:64, :], rb[:])
                else:
                    nc.gpsimd.tensor_mul(xt[:], pvsbs[h][0:64, :], rb[:])
                xh.append(xt)
            # output projection split by head (contraction halves accumulate)
            for ot in range(KC):
                pool, tag = (ps_proj, "proj") if ot % 2 == 0 else (ps_s, "sg")
                ps = pool.tile([128, QT], f32, tag=tag, name="tps")
                for h in range(HPC):
                    w = wo_sb if h == 0 else woL_sb
                    nc.tensor.matmul(
                        ps[:],
                        w[0:64, ot * 128 : (ot + 1) * 128],
                        xh[h][:],
                        start=(h == 0), stop=(h == 1),
                    )
                ob = opool.tile([128, QT], bf16, tag="o", name="tob")
                if ot % 2 == 0:
                    nc.vector.tensor_copy(ob[:], ps[:])
                else:
                    nc.scalar.copy(ob[:], ps[:])
                nc.sync.dma_start(
                    outT[
                        ot * 128 : (ot + 1) * 128,
                        b * N + qt * QT : b * N + (qt + 1) * QT,
                    ],
                    ob[:],
                )

        # ================= the schedule =================
        # Weight DMAs are issued in first-use order, interleaved with the
        # direct projection sections so the first x tile is never queued
        # behind weights it doesn't need yet.
        # K/V tiles 0,1 (xw=512 -> keys 0:1024) and Q tile 0 run directly so
        # attention(0, qt0) can start as soon as ~3MB of x has landed.
        nc.sync.dma_start(wk_sb[:], wk)
        nc.sync.dma_start(bk_sb[:], bk)
        run_gen(gen_proj(0, kTs[0], xk, wk_sb, bk_sb, 512, 0, 2, split=True))
        nc.sync.dma_start(wv_sb[:], wv)
        nc.sync.dma_start(bv_sb[:], bv)
        run_gen(gen_proj(0, vTs[0], xv, wv_sb, bv_sb, 512, 0, 2, split=True,
                         vtile=vs[0]))
        nc.sync.dma_start(wq_sb[:], wq)
        nc.sync.dma_start(bq_sb[:], bq)
        run_gen(gen_proj(0, qTs[0], xq, wq_sb, bq_sb, 512, 0, 1, split=True))
        nc.sync.dma_start(wo_sb[:], wo)
        nc.sync.dma_start(woL_sb[:], wo[64:128, :])

        fill_q.extend(gen_proj(0, kTs[0], xk, wk_sb, bk_sb, 512, 2, 4))
        fill_q.append("K0")
        fill_q.extend(gen_proj(0, vTs[0], xv, wv_sb, bv_sb, 512, 2, 4,
                               vtile=vs[0]))
        fill_q.append("V0")
        for t in range(1, 4):
            fill_q.extend(gen_proj(0, qTs[0], xq, wq_sb, bq_sb, 512, t, t + 1))
            fill_q.append(f"Q0_{t}")
        fill_q.extend(gen_proj(1, kTs[1], xk, wk_sb, bk_sb, 1024, 0, 2))
        fill_q.append("K1")
        fill_q.extend(gen_proj(1, vTs[1], xv, wv_sb, bv_sb, 1024, 0, 2,
                               vtile=vs[1]))
        fill_q.append("V1")
        for t in range(2):
            fill_q.extend(gen_proj(1, qTs[1], xq, wq_sb, bq_sb, 1024, t, t + 1))
            fill_q.append(f"Q1_{t}")

        for qt in range(NQT):
            if qt > 0:
                drain_to(f"Q0_{qt}")
            pvs = attention_qt(
                0, qt, budget=600,
                kneed={8: "K0"} if qt == 0 else None,
                vneed={8: "V0"} if qt == 0 else None,
            )
            normalize(0, qt, pvs)
            fill_q.extend(gen_outproj(0, qt))

        for qt in range(NQT):
            if qt > 0:
                drain_to(f"Q1_{(qt * QT) // 1024}")
            pvs = attention_qt(
                1, qt, budget=450,
                kneed={0: "K1"} if qt == 0 else None,
                vneed={0: "V1"} if qt == 0 else None,
            )
            if qt < NQT - 1:
                normalize(1, qt, pvs)
                fill_q.extend(gen_outproj(1, qt))
            else:
                flush()
                tail_qt(1, qt, pvs)

    nc.compile()
    _cache["nc"] = nc
    return nc


def kernel(x_q, x_k, x_v, Wq, bq, Wk, bk, Wv, bv, Wo, bo, _trace=False):
    x_q = np.asarray(x_q, dtype=np.float32)
    x_k = np.asarray(x_k, dtype=np.float32)
    x_v = np.asarray(x_v, dtype=np.float32)
    Wq, Wk, Wv, Wo = (np.asarray(w, dtype=np.float32) for w in (Wq, Wk, Wv, Wo))
    bq, bk, bv, bo = (np.asarray(v, dtype=np.float32) for v in (bq, bk, bv, bo))

    bf = ml_dtypes.bfloat16
    xqT = np.ascontiguousarray(x_q.reshape(R, D).T).astype(bf)
    xkT = np.ascontiguousarray(x_k.reshape(R, D).T).astype(bf)
    xvT = np.ascontiguousarray(x_v.reshape(R, D).T).astype(bf)

    in_maps = []
    for c in range(NC):
        s = slice(DHC * c, DHC * (c + 1))
        in_maps.append(
            {
                "xqT": xqT,
                "xkT": xkT,
                "xvT": xvT,
                "wqT": _foldw(Wq[s, :].T).astype(bf),
                "wkT": _foldw(Wk[s, :].T).astype(bf),
                "wvT": _foldw(Wv[s, :].T).astype(bf),
                "woT": np.ascontiguousarray(Wo[:, s].T).astype(bf),
                "bq": bq[s][:, None].copy(),
                "bk": bk[s][:, None].copy(),
                "bv": bv[s][:, None].copy(),
            }
        )

    nc = build()
    res = run_bass_kernel_spmd(nc, in_maps, core_ids=list(range(NC)), trace=_trace)
    total = np.zeros((D, R), dtype=np.float32)
    for c in range(NC):
        total += res.results[c]["outT"].astype(np.float32)
    out = total.T + bo[None, :]
    if _trace:
        kernel.last_exec_time_ns = res.exec_time_ns
    return out.reshape(B, N, D).astype(np.float32)


# revision 24
# speedup vs baseline: 1.0138x; 1.0138x over previous
"""Multi-head attention (B=2, N=2048, D=1024, H=16) on 8 TRN2 NeuronCores.

Sharding: tensor-parallel over heads. Core c owns heads 2c, 2c+1 (a 128-wide
slice of the concat head dim). Each core:
  - projects Q^T, K^T, V^T (transposed layout [dh, rows]) for its heads over
    all B*N=4096 rows from host-transposed bf16 x^T inputs (W stationary,
    x moving -> all matmuls stream 512 moving cols)
  - V^T is PE-transposed (identity matmul) into natural [keys, dh] tiles with
    an appended ones-column per head (sumexp rides the PV matmul for free)
  - attention with transposed scores S^T[k, q] = K Q^T, exp on ScalarE
    (scale=1/8 folded in; no max-subtract needed: |scores/8| < ~4)
  - partial output projection out^T_c = Wo[:, slice] X_c^T -> [1024, 4096]
Host sums the 8 partial outputs and adds bo.

Schedule: one software-pipelined instruction stream. The attention kt loop is
scalar-bound (exp ~1.1us/kt vs ~0.86us of PE work), so all other PE work
(remaining projections, output projections) is queued as "filler" matmuls and
pumped between the scores and PV matmuls of each kt iteration; sentinels in
the filler queue let attention start as soon as the first row-tiles of K/V/Q
are projected (exp starts ~15us instead of ~50us). The PE stays continuously
busy, which also keeps the HAM clock-gate at full rate. PSUM: sg ring 2x2
banks + pv ring 2 banks + proj/out ring 2 banks = 8 banks exactly. The tail
(last q-tile) uses a PE-selector matmul instead of the slow SBUF-DMA shift
for the sumexp row, splits its output projection per head to skip the xT
assembly DMA, and runs junk warm-up matmuls through the normalize chain so
the final matmuls are not clock-gated.
"""

import sys

sys.path.insert(0, "/opt/trn_rl_repo")

from contextlib import ExitStack

import ml_dtypes
import numpy as np

import concourse.bass as bass
import concourse.mybir as mybir
import concourse.tile as tile
from concourse import bacc, masks
from concourse.bass_utils import run_bass_kernel_spmd

B, N, D, H, DH = 2, 2048, 1024, 16, 64
R = B * N  # 4096
NC = 8
HPC = H // NC  # 2 heads per core
DHC = HPC * DH  # 128 head dims per core
QT = 512  # query tile (psum bank of fp32)
KT = 128  # key tile (psum partitions)
NQT = N // QT  # 4
NKT = N // KT  # 16
KC = D // 128  # 8 contraction chunks

f32 = mybir.dt.float32
bf16 = mybir.dt.bfloat16

_cache = {}


def _fold(ap):
    # [D, X] dram -> [128, KC, X] partition-folded view for one-shot DMA
    return ap.rearrange("(a p) m -> p a m", p=128)


def _foldw(w):
    # [D, DHC] host weight -> [128, KC, DHC] partition-folded, contiguous
    return np.ascontiguousarray(w.reshape(KC, 128, DHC).transpose(1, 0, 2))


def build():
    if "nc" in _cache:
        return _cache["nc"]
    nc = bacc.Bacc("TRN2", target_bir_lowering=False, debug=False, num_devices=NC)
    xq = nc.dram_tensor("xqT", [D, R], bf16, kind="ExternalInput").ap()
    xk = nc.dram_tensor("xkT", [D, R], bf16, kind="ExternalInput").ap()
    xv = nc.dram_tensor("xvT", [D, R], bf16, kind="ExternalInput").ap()
    wq = nc.dram_tensor("wqT", [128, KC, DHC], bf16, kind="ExternalInput").ap()
    wk = nc.dram_tensor("wkT", [128, KC, DHC], bf16, kind="ExternalInput").ap()
    wv = nc.dram_tensor("wvT", [128, KC, DHC], bf16, kind="ExternalInput").ap()
    wo = nc.dram_tensor("woT", [DHC, D], bf16, kind="ExternalInput").ap()
    bq = nc.dram_tensor("bq", [DHC, 1], f32, kind="ExternalInput").ap()
    bk = nc.dram_tensor("bk", [DHC, 1], f32, kind="ExternalInput").ap()
    bv = nc.dram_tensor("bv", [DHC, 1], f32, kind="ExternalInput").ap()
    outT = nc.dram_tensor("outT", [D, R], bf16, kind="ExternalOutput").ap()

    with tile.TileContext(nc) as tc, ExitStack() as ctx:
        const = ctx.enter_context(tc.tile_pool(name="const", bufs=1))
        xpool = ctx.enter_context(tc.tile_pool(name="x", bufs=3))
        big = ctx.enter_context(tc.tile_pool(name="big", bufs=1))
        ppool = ctx.enter_context(tc.tile_pool(name="p", bufs=3))
        opool = ctx.enter_context(tc.tile_pool(name="o", bufs=4))
        npool = ctx.enter_context(tc.tile_pool(name="norm", bufs=2))
        ps_proj = ctx.enter_context(tc.tile_pool(name="psA", bufs=2, space="PSUM"))
        ps_s = ctx.enter_context(tc.tile_pool(name="psS", bufs=2, space="PSUM"))
        ps_pv = ctx.enter_context(tc.tile_pool(name="psPV", bufs=2, space="PSUM"))

        # ---- local SBUF constants / persistent tiles ----
        warm = const.tile([128, QT], bf16, tag="warm")
        nc.gpsimd.memset(warm[:], 0.0)
        ident = const.tile([128, 128], bf16, tag="ident")
        masks.make_identity(nc, ident[:])
        # f32 selector column: picks the sumexp row (partition 64) via matmul
        sel = const.tile([65, 1], f32, tag="sel")
        nc.vector.memset(sel[:], 0.0)
        nc.vector.memset(sel[64:65, :], 1.0)
        # f32 ones row: partition-broadcast via PE (tail normalize)
        ones64 = const.tile([1, 64], f32, tag="ones64")
        nc.vector.memset(ones64[:], 1.0)

        wk_sb = const.tile([128, KC, DHC], bf16, tag="wk")
        bk_sb = const.tile([DHC, 1], f32, tag="bk")
        wv_sb = const.tile([128, KC, DHC], bf16, tag="wv")
        bv_sb = const.tile([DHC, 1], f32, tag="bv")
        wq_sb = const.tile([128, KC, DHC], bf16, tag="wq")
        bq_sb = const.tile([DHC, 1], f32, tag="bq")
        wo_sb = const.tile([128, D], bf16, tag="wo")
        # partition-0 copy of Wo's h1 rows for the tail's per-head split
        woL_sb = const.tile([64, D], bf16, tag="woL")

        qTs, kTs, vTs, vs, xTs = [], [], [], [], []
        for b in range(B):
            qTs.append(big.tile([128, N], bf16, tag=f"qT{b}", name=f"qT{b}"))
            kTs.append(big.tile([128, N], bf16, tag=f"kT{b}", name=f"kT{b}"))
            vTs.append(big.tile([128, N], bf16, tag=f"vT{b}", name=f"vT{b}"))
            v = big.tile([128, NKT, HPC, 65], bf16, tag=f"v{b}", name=f"v{b}")
            nc.gpsimd.memset(v[:, :, :, 64:65], 1.0)
            vs.append(v)
            xTs.append(opool.tile([128, N], bf16, tag=f"xT{b}", name=f"xT{b}"))

        # ---- PE warm-up: garbage matmuls lift the HAM clock-gate while the
        # first DMAs land (outputs never read) ----
        def warm_mms(n, pool, tag):
            for _ in range(n):
                psw = pool.tile([65, QT], f32, tag=tag, name="psw")
                nc.tensor.matmul(
                    psw[:], warm[:, 0:65], warm[:], start=True, stop=True
                )

        warm_mms(8, ps_proj, "proj")

        # ---- preload the exp activation table while ScalarE is idle ----
        junk = const.tile([128, 1], bf16, tag="junk")
        nc.scalar.activation(
            junk[:], warm[:, 0:1], mybir.ActivationFunctionType.Exp, scale=0.125
        )

        # ================= filler machinery =================
        fill_q = []
        passed = set()

        def pump(ns):
            while fill_q and ns > 0:
                it = fill_q.pop(0)
                if isinstance(it, str):
                    passed.add(it)
                    continue
                ns -= it()

        def drain_to(tag):
            if tag in passed:
                return
            while fill_q:
                it = fill_q.pop(0)
                if isinstance(it, str):
                    passed.add(it)
                    if it == tag:
                        return
                    continue
                it()

        def flush():
            while fill_q:
                it = fill_q.pop(0)
                if isinstance(it, str):
                    passed.add(it)
                else:
                    it()

        def drain_free():
            # emit leading zero-cost ops (DMA issues) without touching matmuls
            while fill_q and (
                isinstance(fill_q[0], str) or getattr(fill_q[0], "free", False)
            ):
                it = fill_q.pop(0)
                if isinstance(it, str):
                    passed.add(it)
                else:
                    it()

        def run_gen(g):
            for f in g:
                f()

        # ---- projection generator: dst[dh, rows] = W^T x^T (+ bias) ----
        # yields micro-ops; V transposes trail their chunk by one chunk so the
        # PE never waits on the bias-add.
        def gen_proj(b, dst, xdram, w_sb, b_sb, xw, t0, t1, split=False,
                     vtile=None):
            pend = []
            for xi in range(t0, t1):
                rlo = b * N + xi * xw
                xt = xpool.tile([128, KC, 1024], bf16, tag="x", name="xt")
                src = _fold(xdram[:, rlo : rlo + xw])
                if split:
                    def d1(xt=xt, src=src, xw=xw):
                        nc.sync.dma_start(xt[:, 0:1, 0:xw], src[:, 0:1, :])
                        return 0
                    d1.free = True
                    yield d1
                    def d2(xt=xt, src=src, xw=xw):
                        nc.sync.dma_start(xt[:, 1:KC, 0:xw], src[:, 1:KC, :])
                        return 0
                    d2.free = True
                    yield d2
                else:
                    def d0(xt=xt, src=src, xw=xw):
                        nc.sync.dma_start(xt[:, :, 0:xw], src)
                        return 0
                    d0.free = True
                    yield d0
                for rl in range(xw // QT):
                    ps = ps_proj.tile([128, QT], f32, tag="proj", name="psp")
                    for kc in range(KC):
                        def mm(ps=ps, xt=xt, kc=kc, rl=rl):
                            nc.tensor.matmul(
                                ps[:],
                                w_sb[:, kc, :],
                                xt[:, kc, rl * QT : (rl + 1) * QT],
                                start=(kc == 0), stop=(kc == KC - 1),
                            )
                            return 215
                        yield mm
                    rt = (xi * xw + rl * QT) // QT
                    def badd(ps=ps, rt=rt):
                        nc.vector.tensor_scalar_add(
                            dst[:, rt * QT : (rt + 1) * QT], ps[:], b_sb[:]
                        )
                        return 0
                    yield badd
                    yield from pend
                    pend = []
                    if vtile is not None:
                        for kt in range(rt * 4, rt * 4 + 4):
                            pst = ps_proj.tile(
                                [128, 128], bf16, tag="proj", name="tp"
                            )
                            def tr(kt=kt, pst=pst):
                                nc.tensor.transpose(
                                    pst[:],
                                    dst[:, kt * 128 : (kt + 1) * 128],
                                    ident[:],
                                )
                                return 215
                            pend.append(tr)
                            def trc(kt=kt, pst=pst, vtile=vtile):
                                nc.vector.tensor_copy(
                                    vtile[:, kt, :, 0:64],
                                    pst[:].rearrange("p (h c) -> p h c", h=HPC),
                                )
                                return 0
                            pend.append(trc)
            yield from pend

        # ---- output projection generator for one (b, qt) ----
        def gen_outproj(b, qt):
            qs = slice(qt * QT, (qt + 1) * QT)
            for ot in range(KC):
                ps = ps_proj.tile([128, QT], f32, tag="proj", name="pso")
                def mm(ps=ps, ot=ot, b=b):
                    nc.tensor.matmul(
                        ps[:],
                        wo_sb[:, ot * 128 : (ot + 1) * 128],
                        xTs[b][:, qs],
                        start=True, stop=True,
                    )
                    return 215
                yield mm
                ob = opool.tile([128, QT], bf16, tag="o", name="ob")
                def cst(ps=ps, ob=ob):
                    nc.vector.tensor_copy(ob[:], ps[:])
                    return 0
                yield cst
                def st(ob=ob, ot=ot, b=b, qt=qt):
                    nc.sync.dma_start(
                        outT[
                            ot * 128 : (ot + 1) * 128,
                            b * N + qt * QT : b * N + (qt + 1) * QT,
                        ],
                        ob[:],
                    )
                    return 0
                yield st

        # ---- attention for one (b, qt): software-pipelined kt loop ----
        def attention_qt(b, qt, budget, kneed=None, vneed=None):
            drain_free()
            qs = slice(qt * QT, (qt + 1) * QT)
            pvs = [
                ps_pv.tile([65, QT], f32, tag="pv", name=f"pv{h}")
                for h in range(HPC)
            ]
            sgs, pts = {}, {}

            def scores(kt):
                if kneed and kt in kneed:
                    drain_to(kneed[kt])
                sg = ps_s.tile([128, 2 * QT], f32, tag="sg", name="sg")
                sgs[kt] = sg
                ks = slice(kt * KT, (kt + 1) * KT)
                for h in range(HPC):
                    hp = slice(64 * h, 64 * h + 64)
                    nc.tensor.matmul(
                        sg[:, h * QT : (h + 1) * QT],
                        kTs[b][hp, ks],
                        qTs[b][hp, qs],
                        start=True, stop=True,
                    )

            def expx(kt):
                pt = ppool.tile([128, 2 * QT], bf16, tag="pt", name="pt")
                pts[kt] = pt
                nc.scalar.activation(
                    pt[:], sgs.pop(kt)[:],
                    mybir.ActivationFunctionType.Exp, scale=0.125,
                )

            def pv(kt):
                if vneed and kt in vneed:
                    drain_to(vneed[kt])
                pt = pts.pop(kt)
                for h in range(HPC):
                    nc.tensor.matmul(
                        pvs[h][:],
                        vs[b][:, kt, h, :],
                        pt[:, h * QT : (h + 1) * QT],
                        start=(kt == 0), stop=(kt == NKT - 1),
                    )

            scores(0)
            expx(0)
            for kt in range(NKT):
                if kt + 1 < NKT:
                    scores(kt + 1)
                    expx(kt + 1)
                if fill_q:
                    pump(budget)
                else:
                    # no filler work: issue a junk matmul so the PE's duty
                    # cycle stays above the HAM clock-gate threshold
                    warm_mms(1, ps_proj, "proj")
                pv(kt)
            return pvs

        # ---- normalize X = PV / sumexp into xTs[b][:, qs] ----
        def normalize(b, qt, pvs):
            qs = slice(qt * QT, (qt + 1) * QT)
            for h in range(HPC):
                pvsb = npool.tile([65, QT], f32, tag="pvsb", name=f"pvsb{h}")
                nc.vector.tensor_copy(pvsb[:], pvs[h][:])
                rc = npool.tile([1, QT], f32, tag="rc", name=f"rc{h}")
                nc.sync.dma_start(rc[:], pvsb[64:65, :])
                nc.vector.reciprocal_approx_fast(rc[:], rc[:])
                rb = npool.tile([64, QT], f32, tag="rb", name=f"rb{h}")
                nc.gpsimd.partition_broadcast(rb[:], rc[:])
                if h == 0:
                    nc.vector.tensor_mul(xTs[b][0:64, qs], pvsb[0:64, :], rb[:])
                else:
                    tmp = npool.tile([64, QT], bf16, tag="tmp", name="tmp")
                    nc.vector.tensor_mul(tmp[:], pvsb[0:64, :], rb[:])
                    nc.sync.dma_start(xTs[b][64:128, qs], tmp[:])

        # ---- tail: last q-tile normalize + per-head-split output proj ----
        # gpsimd-free (its queue may sit behind store drains): the sumexp row
        # shift and the partition broadcast both run as PE matmuls.
        def tail_qt(b, qt, pvs):
            qs = slice(qt * QT, (qt + 1) * QT)
            xh, zr = [], []
            # copies first (h1 on ScalarE so the two heads run in parallel)
            pvsbs = []
            for h in range(HPC):
                pvsb = npool.tile([65, QT], f32, tag="pvsb", name=f"tpv{h}")
                if h == 0:
                    nc.vector.tensor_copy(pvsb[:], pvs[h][:])
                else:
                    nc.scalar.copy(pvsb[:], pvs[h][:])
                pvsbs.append(pvsb)
            # z-row shift via PE selector matmul (partition 64 -> 0)
            for h in range(HPC):
                zp = ps_pv.tile([1, QT], f32, tag="pv", name=f"zp{h}")
                nc.tensor.matmul(
                    zp[:], sel[:], pvsbs[h][:], start=True, stop=True
                )
                zr.append(zp)
            # junk matmuls keep the PE busy (HAM stays at full clock) while
            # the reciprocals run on DVE
            warm_mms(4, ps_s, "sg")
            rcs = []
            for h in range(HPC):
                rc = npool.tile([1, QT], f32, tag="rc", name=f"trc{h}")
                nc.vector.reciprocal_approx_fast(rc[:], zr[h][:])
                rcs.append(rc)
            # broadcast 1/z across 64 partitions via PE outer product
            rbs = []
            for h in range(HPC):
                rb = ps_pv.tile([64, QT], f32, tag="pv", name=f"trb{h}")
                nc.tensor.matmul(rb[:], ones64[:], rcs[h][:], start=True, stop=True)
                rbs.append(rb)
            warm_mms(4, ps_s, "sg")
            for h in range(HPC):
                xt = npool.tile([64, QT], bf16, tag="tmp", name=f"tx{h}")
                nc.vector.tensor_mul(xt[:], pvsbs[h][0:64, :], rbs[h][:])
                xh.append(xt)
            # output projection split by head (contraction halves accumulate)
            for ot in range(KC):
                pool, tag = (ps_proj, "proj") if ot % 2 == 0 else (ps_s, "sg")
                ps = pool.tile([128, QT], f32, tag=tag, name="tps")
                for h in range(HPC):
                    w = wo_sb if h == 0 else woL_sb
                    nc.tensor.matmul(
                        ps[:],
                        w[0:64, ot * 128 : (ot + 1) * 128],
                        xh[h][:],
                        start=(h == 0), stop=(h == 1),
                    )
                ob = opool.tile([128, QT], bf16, tag="o", name="tob")
                if ot % 2 == 0:
                    nc.vector.tensor_copy(ob[:], ps[:])
                else:
                    nc.scalar.copy(ob[:], ps[:])
                nc.sync.dma_start(
                    outT[
                        ot * 128 : (ot + 1) * 128,
                        b * N + qt * QT : b * N + (qt + 1) * QT,
                    ],
                    ob[:],
                )

        # ================= the schedule =================
        # Weight DMAs are issued in first-use order, interleaved with the
        # direct projection sections so the first x tile is never queued
        # behind weights it doesn't need yet.
        # K/V tiles 0,1 (xw=512 -> keys 0:1024) and Q tile 0 run directly so
        # attention(0, qt0) can start as soon as ~3MB of x has landed.
        nc.sync.dma_start(wk_sb[:], wk)
        nc.sync.dma_start(bk_sb[:], bk)
        run_gen(gen_proj(0, kTs[0], xk, wk_sb, bk_sb, 512, 0, 2, split=True))
        nc.sync.dma_start(wq_sb[:], wq)
        nc.sync.dma_start(bq_sb[:], bq)
        run_gen(gen_proj(0, qTs[0], xq, wq_sb, bq_sb, 512, 0, 1, split=True))
        nc.sync.dma_start(wv_sb[:], wv)
        nc.sync.dma_start(bv_sb[:], bv)
        nc.sync.dma_start(wo_sb[:], wo)
        nc.sync.dma_start(woL_sb[:], wo[64:128, :])

        for t in range(2):
            fill_q.extend(gen_proj(0, vTs[0], xv, wv_sb, bv_sb, 512, t, t + 1,
                                   split=(t == 0), vtile=vs[0]))
            fill_q.append(f"V0{'ab'[t]}")
        fill_q.extend(gen_proj(0, kTs[0], xk, wk_sb, bk_sb, 512, 2, 4))
        fill_q.append("K0")
        fill_q.extend(gen_proj(0, vTs[0], xv, wv_sb, bv_sb, 512, 2, 4,
                               vtile=vs[0]))
        fill_q.append("V0c")
        for t in range(1, 4):
            fill_q.extend(gen_proj(0, qTs[0], xq, wq_sb, bq_sb, 512, t, t + 1))
            fill_q.append(f"Q0_{t}")
        fill_q.extend(gen_proj(1, kTs[1], xk, wk_sb, bk_sb, 1024, 0, 2))
        fill_q.append("K1")
        fill_q.extend(gen_proj(1, vTs[1], xv, wv_sb, bv_sb, 1024, 0, 2,
                               vtile=vs[1]))
        fill_q.append("V1")
        for t in range(2):
            fill_q.extend(gen_proj(1, qTs[1], xq, wq_sb, bq_sb, 1024, t, t + 1))
            fill_q.append(f"Q1_{t}")

        for qt in range(NQT):
            if qt > 0:
                drain_to(f"Q0_{qt}")
            pvs = attention_qt(
                0, qt, budget=600,
                kneed={8: "K0"} if qt == 0 else None,
                vneed={0: "V0a", 4: "V0b", 8: "V0c"} if qt == 0 else None,
            )
            normalize(0, qt, pvs)
            fill_q.extend(gen_outproj(0, qt))

        for qt in range(NQT):
            if qt > 0:
                drain_to(f"Q1_{(qt * QT) // 1024}")
            pvs = attention_qt(
                1, qt, budget=450,
                kneed={0: "K1"} if qt == 0 else None,
                vneed={0: "V1"} if qt == 0 else None,
            )
            if qt < NQT - 1:
                normalize(1, qt, pvs)
                fill_q.extend(gen_outproj(1, qt))
            else:
                flush()
                tail_qt(1, qt, pvs)

    nc.compile()
    _cache["nc"] = nc
    return nc


def kernel(x_q, x_k, x_v, Wq, bq, Wk, bk, Wv, bv, Wo, bo, _trace=False):
    x_q = np.asarray(x_q, dtype=np.float32)
    x_k = np.asarray(x_k, dtype=np.float32)
    x_v = np.asarray(x_v, dtype=np.float32)
    Wq, Wk, Wv, Wo = (np.asarray(w, dtype=np.float32) for w in (Wq, Wk, Wv, Wo))
    bq, bk, bv, bo = (np.asarray(v, dtype=np.float32) for v in (bq, bk, bv, bo))

    bf = ml_dtypes.bfloat16
    xqT = np.ascontiguousarray(x_q.reshape(R, D).T).astype(bf)
    xkT = np.ascontiguousarray(x_k.reshape(R, D).T).astype(bf)
    xvT = np.ascontiguousarray(x_v.reshape(R, D).T).astype(bf)

    in_maps = []
    for c in range(NC):
        s = slice(DHC * c, DHC * (c + 1))
        in_maps.append(
            {
                "xqT": xqT,
                "xkT": xkT,
                "xvT": xvT,
                "wqT": _foldw(Wq[s, :].T).astype(bf),
                "wkT": _foldw(Wk[s, :].T).astype(bf),
                "wvT": _foldw(Wv[s, :].T).astype(bf),
                "woT": np.ascontiguousarray(Wo[:, s].T).astype(bf),
                "bq": bq[s][:, None].copy(),
                "bk": bk[s][:, None].copy(),
                "bv": bv[s][:, None].copy(),
            }
        )

    nc = build()
    res = run_bass_kernel_spmd(nc, in_maps, core_ids=list(range(NC)), trace=_trace)
    total = np.zeros((D, R), dtype=np.float32)
    for c in range(NC):
        total += res.results[c]["outT"].astype(np.float32)
    out = total.T + bo[None, :]
    if _trace:
        kernel.last_exec_time_ns = res.exec_time_ns
    return out.reshape(B, N, D).astype(np.float32)


# revision 29
# speedup vs baseline: 1.0199x; 1.0060x over previous
"""Multi-head attention (B=2, N=2048, D=1024, H=16) on 8 TRN2 NeuronCores.

Sharding: tensor-parallel over heads. Core c owns heads 2c, 2c+1 (a 128-wide
slice of the concat head dim). Each core:
  - projects Q^T, K^T, V^T (transposed layout [dh, rows]) for its heads over
    all B*N=4096 rows from host-transposed bf16 x^T inputs (W stationary,
    x moving -> all matmuls stream 512 moving cols)
  - V^T is PE-transposed (identity matmul) into natural [keys, dh] tiles with
    an appended ones-column per head (sumexp rides the PV matmul for free)
  - attention with transposed scores S^T[k, q] = K Q^T, exp on ScalarE
    (scale=1/8 folded in; no max-subtract needed: |scores/8| < ~4)
  - partial output projection out^T_c = Wo[:, slice] X_c^T -> [1024, 4096]
Host sums the 8 partial outputs and adds bo.

Schedule: one software-pipelined instruction stream. The attention kt loop is
scalar-bound (exp ~1.1us/kt vs ~0.86us of PE work), so all other PE work
(remaining projections, output projections) is queued as "filler" matmuls and
pumped between the scores and PV matmuls of each kt iteration; sentinels in
the filler queue let attention start as soon as the first row-tiles of K/V/Q
are projected (exp starts ~15us instead of ~50us). The PE stays continuously
busy, which also keeps the HAM clock-gate at full rate. PSUM: sg ring 2x2
banks + pv ring 2 banks + proj/out ring 2 banks = 8 banks exactly. The tail
(last q-tile) uses a PE-selector matmul instead of the slow SBUF-DMA shift
for the sumexp row, splits its output projection per head to skip the xT
assembly DMA, and runs junk warm-up matmuls through the normalize chain so
the final matmuls are not clock-gated.
"""

import sys

sys.path.insert(0, "/opt/trn_rl_repo")

from contextlib import ExitStack

import ml_dtypes
import numpy as np

import concourse.bass as bass
import concourse.mybir as mybir
import concourse.tile as tile
from concourse import bacc, masks
from concourse.bass_utils import run_bass_kernel_spmd

B, N, D, H, DH = 2, 2048, 1024, 16, 64
R = B * N  # 4096
NC = 8
HPC = H // NC  # 2 heads per core
DHC = HPC * DH  # 128 head dims per core
QT = 512  # query tile (psum bank of fp32)
KT = 128  # key tile (psum partitions)
NQT = N // QT  # 4
NKT = N // KT  # 16
KC = D // 128  # 8 contraction chunks

f32 = mybir.dt.float32
bf16 = mybir.dt.bfloat16

_cache = {}


def _fold(ap):
    # [D, X] dram -> [128, KC, X] partition-folded view for one-shot DMA
    return ap.rearrange("(a p) m -> p a m", p=128)


def _foldw(w):
    # [D, DHC] host weight -> [128, KC, DHC] partition-folded, contiguous
    return np.ascontiguousarray(w.reshape(KC, 128, DHC).transpose(1, 0, 2))


def build():
    if "nc" in _cache:
        return _cache["nc"]
    nc = bacc.Bacc("TRN2", target_bir_lowering=False, debug=False, num_devices=NC)
    xq = nc.dram_tensor("xqT", [D, R], bf16, kind="ExternalInput").ap()
    xk = nc.dram_tensor("xkT", [D, R], bf16, kind="ExternalInput").ap()
    xv = nc.dram_tensor("xvT", [D, R], bf16, kind="ExternalInput").ap()
    wq = nc.dram_tensor("wqT", [128, KC, DHC], bf16, kind="ExternalInput").ap()
    wk = nc.dram_tensor("wkT", [128, KC, DHC], bf16, kind="ExternalInput").ap()
    wv = nc.dram_tensor("wvT", [128, KC, DHC], bf16, kind="ExternalInput").ap()
    wo = nc.dram_tensor("woT", [DHC, D], bf16, kind="ExternalInput").ap()
    bq = nc.dram_tensor("bq", [DHC, 1], f32, kind="ExternalInput").ap()
    bk = nc.dram_tensor("bk", [DHC, 1], f32, kind="ExternalInput").ap()
    bv = nc.dram_tensor("bv", [DHC, 1], f32, kind="ExternalInput").ap()
    outT = nc.dram_tensor("outT", [D, R], bf16, kind="ExternalOutput").ap()

    with tile.TileContext(nc) as tc, ExitStack() as ctx:
        const = ctx.enter_context(tc.tile_pool(name="const", bufs=1))
        xpool = ctx.enter_context(tc.tile_pool(name="x", bufs=3))
        big = ctx.enter_context(tc.tile_pool(name="big", bufs=1))
        ppool = ctx.enter_context(tc.tile_pool(name="p", bufs=3))
        opool = ctx.enter_context(tc.tile_pool(name="o", bufs=4))
        npool = ctx.enter_context(tc.tile_pool(name="norm", bufs=2))
        ps_proj = ctx.enter_context(tc.tile_pool(name="psA", bufs=2, space="PSUM"))
        ps_s = ctx.enter_context(tc.tile_pool(name="psS", bufs=2, space="PSUM"))
        ps_pv = ctx.enter_context(tc.tile_pool(name="psPV", bufs=2, space="PSUM"))

        # ---- local SBUF constants / persistent tiles ----
        warm = const.tile([128, QT], bf16, tag="warm")
        nc.gpsimd.memset(warm[:], 0.0)
        ident = const.tile([128, 128], bf16, tag="ident")
        masks.make_identity(nc, ident[:])
        # f32 selector column: picks the sumexp row (partition 64) via matmul
        sel = const.tile([65, 1], f32, tag="sel")
        nc.vector.memset(sel[:], 0.0)
        nc.vector.memset(sel[64:65, :], 1.0)
        # f32 ones row: partition-broadcast via PE (tail normalize)
        ones64 = const.tile([1, 64], f32, tag="ones64")
        nc.vector.memset(ones64[:], 1.0)

        wk_sb = const.tile([128, KC, DHC], bf16, tag="wk")
        bk_sb = const.tile([DHC, 1], f32, tag="bk")
        wv_sb = const.tile([128, KC, DHC], bf16, tag="wv")
        bv_sb = const.tile([DHC, 1], f32, tag="bv")
        wq_sb = const.tile([128, KC, DHC], bf16, tag="wq")
        bq_sb = const.tile([DHC, 1], f32, tag="bq")
        wo_sb = const.tile([128, D], bf16, tag="wo")
        # partition-0 copy of Wo's h1 rows for the tail's per-head split
        woL_sb = const.tile([64, D], bf16, tag="woL")

        qTs, kTs, vTs, vs, xTs = [], [], [], [], []
        for b in range(B):
            qTs.append(big.tile([128, N], bf16, tag=f"qT{b}", name=f"qT{b}"))
            kTs.append(big.tile([128, N], bf16, tag=f"kT{b}", name=f"kT{b}"))
            vTs.append(big.tile([128, N], bf16, tag=f"vT{b}", name=f"vT{b}"))
            v = big.tile([128, NKT, HPC, 65], bf16, tag=f"v{b}", name=f"v{b}")
            nc.gpsimd.memset(v[:, :, :, 64:65], 1.0)
            vs.append(v)
            xTs.append(opool.tile([128, N], bf16, tag=f"xT{b}", name=f"xT{b}"))

        # ---- PE warm-up: garbage matmuls lift the HAM clock-gate while the
        # first DMAs land (outputs never read) ----
        def warm_mms(n, pool, tag):
            for _ in range(n):
                psw = pool.tile([65, QT], f32, tag=tag, name="psw")
                nc.tensor.matmul(
                    psw[:], warm[:, 0:65], warm[:], start=True, stop=True
                )

        warm_mms(8, ps_proj, "proj")

        # ---- preload the exp activation table while ScalarE is idle ----
        junk = const.tile([128, 1], bf16, tag="junk")
        nc.scalar.activation(
            junk[:], warm[:, 0:1], mybir.ActivationFunctionType.Exp, scale=0.125
        )

        # ================= filler machinery =================
        fill_q = []
        passed = set()

        def pump(ns):
            spent = 0
            while fill_q and ns > spent:
                it = fill_q.pop(0)
                if isinstance(it, str):
                    passed.add(it)
                    continue
                spent += it()
            return spent

        def drain_to(tag):
            if tag in passed:
                return
            while fill_q:
                it = fill_q.pop(0)
                if isinstance(it, str):
                    passed.add(it)
                    if it == tag:
                        return
                    continue
                it()

        def flush():
            while fill_q:
                it = fill_q.pop(0)
                if isinstance(it, str):
                    passed.add(it)
                else:
                    it()

        def drain_free():
            # emit leading zero-cost ops (DMA issues) without touching matmuls
            while fill_q and (
                isinstance(fill_q[0], str) or getattr(fill_q[0], "free", False)
            ):
                it = fill_q.pop(0)
                if isinstance(it, str):
                    passed.add(it)
                else:
                    it()

        def run_gen(g):
            for f in g:
                f()

        # ---- projection generator: dst[dh, rows] = W^T x^T (+ bias) ----
        # yields micro-ops; V transposes trail their chunk by one chunk so the
        # PE never waits on the bias-add.
        def gen_proj(b, dst, xdram, w_sb, b_sb, xw, t0, t1, split=False,
                     vtile=None):
            pend = []
            for xi in range(t0, t1):
                rlo = b * N + xi * xw
                xt = xpool.tile([128, KC, 1024], bf16, tag="x", name="xt")
                src = _fold(xdram[:, rlo : rlo + xw])
                if split:
                    def d1(xt=xt, src=src, xw=xw):
                        nc.sync.dma_start(xt[:, 0:1, 0:xw], src[:, 0:1, :])
                        return 0
                    d1.free = True
                    yield d1
                    def d2(xt=xt, src=src, xw=xw):
                        nc.sync.dma_start(xt[:, 1:KC, 0:xw], src[:, 1:KC, :])
                        return 0
                    d2.free = True
                    yield d2
                else:
                    def d0(xt=xt, src=src, xw=xw):
                        nc.sync.dma_start(xt[:, :, 0:xw], src)
                        return 0
                    d0.free = True
                    yield d0
                for rl in range(xw // QT):
                    ps = ps_proj.tile([128, QT], f32, tag="proj", name="psp")
                    for kc in range(KC):
                        def mm(ps=ps, xt=xt, kc=kc, rl=rl):
                            nc.tensor.matmul(
                                ps[:],
                                w_sb[:, kc, :],
                                xt[:, kc, rl * QT : (rl + 1) * QT],
                                start=(kc == 0), stop=(kc == KC - 1),
                            )
                            return 215
                        yield mm
                    rt = (xi * xw + rl * QT) // QT
                    def badd(ps=ps, rt=rt):
                        nc.vector.tensor_scalar_add(
                            dst[:, rt * QT : (rt + 1) * QT], ps[:], b_sb[:]
                        )
                        return 0
                    yield badd
                    yield from pend
                    pend = []
                    if vtile is not None:
                        for kt in range(rt * 4, rt * 4 + 4):
                            pst = ps_proj.tile(
                                [128, 128], bf16, tag="proj", name="tp"
                            )
                            def tr(kt=kt, pst=pst):
                                nc.tensor.transpose(
                                    pst[:],
                                    dst[:, kt * 128 : (kt + 1) * 128],
                                    ident[:],
                                )
                                return 215
                            pend.append(tr)
                            def trc(kt=kt, pst=pst, vtile=vtile):
                                nc.vector.tensor_copy(
                                    vtile[:, kt, :, 0:64],
                                    pst[:].rearrange("p (h c) -> p h c", h=HPC),
                                )
                                return 0
                            pend.append(trc)
            yield from pend

        # ---- output projection generator for one (b, qt) ----
        def gen_outproj(b, qt):
            qs = slice(qt * QT, (qt + 1) * QT)
            for ot in range(KC):
                ps = ps_proj.tile([128, QT], f32, tag="proj", name="pso")
                def mm(ps=ps, ot=ot, b=b):
                    nc.tensor.matmul(
                        ps[:],
                        wo_sb[:, ot * 128 : (ot + 1) * 128],
                        xTs[b][:, qs],
                        start=True, stop=True,
                    )
                    return 215
                yield mm
                ob = opool.tile([128, QT], bf16, tag="o", name="ob")
                def cst(ps=ps, ob=ob):
                    nc.vector.tensor_copy(ob[:], ps[:])
                    return 0
                yield cst
                def st(ob=ob, ot=ot, b=b, qt=qt):
                    nc.sync.dma_start(
                        outT[
                            ot * 128 : (ot + 1) * 128,
                            b * N + qt * QT : b * N + (qt + 1) * QT,
                        ],
                        ob[:],
                    )
                    return 0
                yield st

        # ---- attention for one (b, qt): software-pipelined kt loop ----
        def attention_qt(b, qt, budget, kneed=None, vneed=None):
            drain_free()
            qs = slice(qt * QT, (qt + 1) * QT)
            pvs = [
                ps_pv.tile([65, QT], f32, tag="pv", name=f"pv{h}")
                for h in range(HPC)
            ]
            sgs, pts = {}, {}

            def scores(kt):
                if kneed and kt in kneed:
                    drain_to(kneed[kt])
                sg = ps_s.tile([128, 2 * QT], f32, tag="sg", name="sg")
                sgs[kt] = sg
                ks = slice(kt * KT, (kt + 1) * KT)
                for h in range(HPC):
                    hp = slice(64 * h, 64 * h + 64)
                    nc.tensor.matmul(
                        sg[:, h * QT : (h + 1) * QT],
                        kTs[b][hp, ks],
                        qTs[b][hp, qs],
                        start=True, stop=True,
                    )

            def expx(kt):
                pt = ppool.tile([128, 2 * QT], bf16, tag="pt", name="pt")
                pts[kt] = pt
                nc.scalar.activation(
                    pt[:], sgs.pop(kt)[:],
                    mybir.ActivationFunctionType.Exp, scale=0.125,
                )

            def pv(kt):
                if vneed and kt in vneed:
                    drain_to(vneed[kt])
                pt = pts.pop(kt)
                for h in range(HPC):
                    nc.tensor.matmul(
                        pvs[h][:],
                        vs[b][:, kt, h, :],
                        pt[:, h * QT : (h + 1) * QT],
                        start=(kt == 0), stop=(kt == NKT - 1),
                    )

            scores(0)
            expx(0)
            for kt in range(NKT):
                if kt + 1 < NKT:
                    scores(kt + 1)
                    expx(kt + 1)
                if pump(budget) < 215:
                    # filler queue dry or starved: junk weight-loads (no psum
                    # needed, no deps) keep the PE's duty cycle above the HAM
                    # clock-gate threshold
                    for _ in range(3):
                        nc.tensor.ldweights(warm[:, 0:128])
                pv(kt)
            return pvs

        # ---- normalize X = PV / sumexp into xTs[b][:, qs] ----
        def normalize(b, qt, pvs):
            qs = slice(qt * QT, (qt + 1) * QT)
            for h in range(HPC):
                pvsb = npool.tile([65, QT], f32, tag="pvsb", name=f"pvsb{h}")
                nc.vector.tensor_copy(pvsb[:], pvs[h][:])
                rc = npool.tile([1, QT], f32, tag="rc", name=f"rc{h}")
                nc.sync.dma_start(rc[:], pvsb[64:65, :])
                nc.vector.reciprocal_approx_fast(rc[:], rc[:])
                rb = npool.tile([64, QT], f32, tag="rb", name=f"rb{h}")
                nc.gpsimd.partition_broadcast(rb[:], rc[:])
                if h == 0:
                    nc.vector.tensor_mul(xTs[b][0:64, qs], pvsb[0:64, :], rb[:])
                else:
                    tmp = npool.tile([64, QT], bf16, tag="tmp", name="tmp")
                    nc.vector.tensor_mul(tmp[:], pvsb[0:64, :], rb[:])
                    nc.sync.dma_start(xTs[b][64:128, qs], tmp[:])

        # ---- tail: last q-tile normalize + per-head-split output proj ----
        # gpsimd-free (its queue may sit behind store drains): the sumexp row
        # shift and the partition broadcast both run as PE matmuls.
        def tail_qt(b, qt, pvs):
            qs = slice(qt * QT, (qt + 1) * QT)
            xh, zr = [], []
            # copies first (h1 on ScalarE so the two heads run in parallel)
            pvsbs = []
            for h in range(HPC):
                pvsb = npool.tile([65, QT], f32, tag="pvsb", name=f"tpv{h}")
                if h == 0:
                    nc.vector.tensor_copy(pvsb[:], pvs[h][:])
                else:
                    nc.scalar.copy(pvsb[:], pvs[h][:])
                pvsbs.append(pvsb)
            # z-row shift via PE selector matmul (partition 64 -> 0)
            for h in range(HPC):
                zp = ps_pv.tile([1, QT], f32, tag="pv", name=f"zp{h}")
                nc.tensor.matmul(
                    zp[:], sel[:], pvsbs[h][:], start=True, stop=True
                )
                zr.append(zp)
            # junk matmuls keep the PE busy (HAM stays at full clock) while
            # the reciprocals run on DVE
            warm_mms(6, ps_s, "sg")
            rcs = []
            for h in range(HPC):
                rc = npool.tile([1, QT], f32, tag="rc", name=f"trc{h}")
                nc.vector.reciprocal_approx_fast(rc[:], zr[h][:])
                rcs.append(rc)
            # broadcast 1/z across 64 partitions via PE outer product
            rbs = []
            for h in range(HPC):
                rb = ps_pv.tile([64, QT], f32, tag="pv", name=f"trb{h}")
                nc.tensor.matmul(rb[:], ones64[:], rcs[h][:], start=True, stop=True)
                rbs.append(rb)
            warm_mms(6, ps_s, "sg")
            for h in range(HPC):
                xt = npool.tile([64, QT], bf16, tag="tmp", name=f"tx{h}")
                nc.vector.tensor_mul(xt[:], pvsbs[h][0:64, :], rbs[h][:])
                xh.append(xt)
            # output projection split by head (contraction halves accumulate)
            for ot in range(KC):
                pool, tag = (ps_proj, "proj") if ot % 2 == 0 else (ps_s, "sg")
                ps = pool.tile([128, QT], f32, tag=tag, name="tps")
                for h in range(HPC):
                    w = wo_sb if h == 0 else woL_sb
                    nc.tensor.matmul(
                        ps[:],
                        w[0:64, ot * 128 : (ot + 1) * 128],
                        xh[h][:],
                        start=(h == 0), stop=(h == 1),
                    )
                ob = opool.tile([128, QT], bf16, tag="o", name="tob")
                if ot % 2 == 0:
                    nc.vector.tensor_copy(ob[:], ps[:])
                else:
                    nc.scalar.copy(ob[:], ps[:])
                # split the final flush across both HWDGE rings
                eng = nc.sync if ot % 2 == 0 else nc.scalar
                eng.dma_start(
                    outT[
                        ot * 128 : (ot + 1) * 128,
                        b * N + qt * QT : b * N + (qt + 1) * QT,
                    ],
                    ob[:],
                )

        # ================= the schedule =================
        # Weight DMAs are issued in first-use order, interleaved with the
        # direct projection sections so the first x tile is never queued
        # behind weights it doesn't need yet.
        # K/V tiles 0,1 (xw=512 -> keys 0:1024) and Q tile 0 run directly so
        # attention(0, qt0) can start as soon as ~3MB of x has landed.
        nc.sync.dma_start(wk_sb[:], wk)
        nc.sync.dma_start(bk_sb[:], bk)
        run_gen(gen_proj(0, kTs[0], xk, wk_sb, bk_sb, 512, 0, 2, split=True))
        nc.sync.dma_start(wq_sb[:], wq)
        nc.sync.dma_start(bq_sb[:], bq)
        run_gen(gen_proj(0, qTs[0], xq, wq_sb, bq_sb, 512, 0, 1, split=True))
        nc.sync.dma_start(wv_sb[:], wv)
        nc.sync.dma_start(bv_sb[:], bv)
        nc.sync.dma_start(wo_sb[:], wo)
        nc.sync.dma_start(woL_sb[:], wo[64:128, :])

        for t in range(2):
            fill_q.extend(gen_proj(0, vTs[0], xv, wv_sb, bv_sb, 512, t, t + 1,
                                   split=(t == 0), vtile=vs[0]))
            fill_q.append(f"V0{'ab'[t]}")
        fill_q.extend(gen_proj(0, kTs[0], xk, wk_sb, bk_sb, 512, 2, 4))
        fill_q.append("K0")
        fill_q.extend(gen_proj(0, vTs[0], xv, wv_sb, bv_sb, 512, 2, 4,
                               vtile=vs[0]))
        fill_q.append("V0c")
        for t in range(1, 4):
            fill_q.extend(gen_proj(0, qTs[0], xq, wq_sb, bq_sb, 512, t, t + 1))
            fill_q.append(f"Q0_{t}")
        fill_q.extend(gen_proj(1, kTs[1], xk, wk_sb, bk_sb, 1024, 0, 2))
        fill_q.append("K1")
        fill_q.extend(gen_proj(1, vTs[1], xv, wv_sb, bv_sb, 1024, 0, 2,
                               vtile=vs[1]))
        fill_q.append("V1")
        for t in range(2):
            fill_q.extend(gen_proj(1, qTs[1], xq, wq_sb, bq_sb, 1024, t, t + 1))
            fill_q.append(f"Q1_{t}")

        for qt in range(NQT):
            if qt > 0:
                drain_to(f"Q0_{qt}")
            pvs = attention_qt(
                0, qt, budget=600,
                kneed={8: "K0"} if qt == 0 else None,
                vneed={0: "V0a", 4: "V0b", 8: "V0c"} if qt == 0 else None,
            )
            normalize(0, qt, pvs)
            fill_q.extend(gen_outproj(0, qt))

        for qt in range(NQT):
            if qt > 0:
                drain_to(f"Q1_{(qt * QT) // 1024}")
            pvs = attention_qt(
                1, qt, budget=450,
                kneed={0: "K1"} if qt == 0 else None,
                vneed={0: "V1"} if qt == 0 else None,
            )
            if qt < NQT - 1:
                normalize(1, qt, pvs)
                fill_q.extend(gen_outproj(1, qt))
            else:
                flush()
                tail_qt(1, qt, pvs)

    nc.compile()
    _cache["nc"] = nc
    return nc


def kernel(x_q, x_k, x_v, Wq, bq, Wk, bk, Wv, bv, Wo, bo, _trace=False):
    x_q = np.asarray(x_q, dtype=np.float32)
    x_k = np.asarray(x_k, dtype=np.float32)
    x_v = np.asarray(x_v, dtype=np.float32)
    Wq, Wk, Wv, Wo = (np.asarray(w, dtype=np.float32) for w in (Wq, Wk, Wv, Wo))
    bq, bk, bv, bo = (np.asarray(v, dtype=np.float32) for v in (bq, bk, bv, bo))

    bf = ml_dtypes.bfloat16
    xqT = np.ascontiguousarray(x_q.reshape(R, D).T).astype(bf)
    xkT = np.ascontiguousarray(x_k.reshape(R, D).T).astype(bf)
    xvT = np.ascontiguousarray(x_v.reshape(R, D).T).astype(bf)

    in_maps = []
    for c in range(NC):
        s = slice(DHC * c, DHC * (c + 1))
        in_maps.append(
            {
                "xqT": xqT,
                "xkT": xkT,
                "xvT": xvT,
                "wqT": _foldw(Wq[s, :].T).astype(bf),
                "wkT": _foldw(Wk[s, :].T).astype(bf),
                "wvT": _foldw(Wv[s, :].T).astype(bf),
                "woT": np.ascontiguousarray(Wo[:, s].T).astype(bf),
                "bq": bq[s][:, None].copy(),
                "bk": bk[s][:, None].copy(),
                "bv": bv[s][:, None].copy(),
            }
        )

    nc = build()
    res = run_bass_kernel_spmd(nc, in_maps, core_ids=list(range(NC)), trace=_trace)
    total = np.zeros((D, R), dtype=np.float32)
    for c in range(NC):
        total += res.results[c]["outT"].astype(np.float32)
    out = total.T + bo[None, :]
    if _trace:
        kernel.last_exec_time_ns = res.exec_time_ns
    return out.reshape(B, N, D).astype(np.float32)


# revision 33
# speedup vs baseline: 1.0734x; 1.0525x over previous
"""Multi-head attention (B=2, N=2048, D=1024, H=16) on 8 TRN2 NeuronCores.

Sharding: tensor-parallel over heads. Core c owns heads 2c, 2c+1 (a 128-wide
slice of the concat head dim). Each core:
  - projects Q^T, K^T, V^T (transposed layout [dh, rows]) for its heads over
    all B*N=4096 rows from host-transposed bf16 x^T inputs (W stationary,
    x moving -> all matmuls stream 512 moving cols)
  - V^T is PE-transposed (identity matmul) into natural [keys, dh] tiles with
    an appended ones-column per head (sumexp rides the PV matmul for free)
  - attention with transposed scores S^T[k, q] = K Q^T, exp on ScalarE
    (scale=1/8 folded in; no max-subtract needed: |scores/8| < ~4)
  - partial output projection out^T_c = Wo[:, slice] X_c^T -> [1024, 4096]
Host sums the 8 partial outputs and adds bo.

Schedule: one software-pipelined instruction stream. The attention kt loop is
scalar-bound (exp ~1.1us/kt vs ~0.86us of PE work), so all other PE work
(remaining projections, output projections) is queued as "filler" matmuls and
pumped between the scores and PV matmuls of each kt iteration; sentinels in
the filler queue let attention start as soon as the first row-tiles of K/V/Q
are projected (exp starts ~15us instead of ~50us). The PE stays continuously
busy, which also keeps the HAM clock-gate at full rate. PSUM: sg ring 2x2
banks + pv ring 2 banks + proj/out ring 2 banks = 8 banks exactly. The tail
(last q-tile) uses a PE-selector matmul instead of the slow SBUF-DMA shift
for the sumexp row, splits its output projection per head to skip the xT
assembly DMA, and runs junk warm-up matmuls through the normalize chain so
the final matmuls are not clock-gated.
"""

import sys

sys.path.insert(0, "/opt/trn_rl_repo")

from contextlib import ExitStack

import ml_dtypes
import numpy as np

import concourse.bass as bass
import concourse.mybir as mybir
import concourse.tile as tile
from concourse import bacc, masks
from concourse.bass_utils import run_bass_kernel_spmd

B, N, D, H, DH = 2, 2048, 1024, 16, 64
R = B * N  # 4096
NC = 8
HPC = H // NC  # 2 heads per core
DHC = HPC * DH  # 128 head dims per core
QT = 512  # query tile (psum bank of fp32)
KT = 128  # key tile (psum partitions)
NQT = N // QT  # 4
NKT = N // KT  # 16
KC = D // 128  # 8 contraction chunks

f32 = mybir.dt.float32
bf16 = mybir.dt.bfloat16

_cache = {}


def _fold(ap):
    # [D, X] dram -> [128, KC, X] partition-folded view for one-shot DMA
    return ap.rearrange("(a p) m -> p a m", p=128)


def _foldw(w):
    # [D, DHC] host weight -> [128, KC, DHC] partition-folded, contiguous
    return np.ascontiguousarray(w.reshape(KC, 128, DHC).transpose(1, 0, 2))


def build():
    if "nc" in _cache:
        return _cache["nc"]
    nc = bacc.Bacc("TRN2", target_bir_lowering=False, debug=False, num_devices=NC)
    xq = nc.dram_tensor("xqT", [D, R], bf16, kind="ExternalInput").ap()
    xk = nc.dram_tensor("xkT", [D, R], bf16, kind="ExternalInput").ap()
    xv = nc.dram_tensor("xvT", [D, R], bf16, kind="ExternalInput").ap()
    wq = nc.dram_tensor("wqT", [128, KC, DHC], bf16, kind="ExternalInput").ap()
    wk = nc.dram_tensor("wkT", [128, KC, DHC], bf16, kind="ExternalInput").ap()
    wv = nc.dram_tensor("wvT", [128, KC, DHC], bf16, kind="ExternalInput").ap()
    wo = nc.dram_tensor("woT", [DHC, D], bf16, kind="ExternalInput").ap()
    bq = nc.dram_tensor("bq", [DHC, 1], f32, kind="ExternalInput").ap()
    bk = nc.dram_tensor("bk", [DHC, 1], f32, kind="ExternalInput").ap()
    bv = nc.dram_tensor("bv", [DHC, 1], f32, kind="ExternalInput").ap()
    outT = nc.dram_tensor("outT", [D, R], bf16, kind="ExternalOutput").ap()

    with tile.TileContext(nc) as tc, ExitStack() as ctx:
        const = ctx.enter_context(tc.tile_pool(name="const", bufs=1))
        xpool = ctx.enter_context(tc.tile_pool(name="x", bufs=3))
        big = ctx.enter_context(tc.tile_pool(name="big", bufs=1))
        ppool = ctx.enter_context(tc.tile_pool(name="p", bufs=3))
        opool = ctx.enter_context(tc.tile_pool(name="o", bufs=4))
        npool = ctx.enter_context(tc.tile_pool(name="norm", bufs=2))
        ps_proj = ctx.enter_context(tc.tile_pool(name="psA", bufs=2, space="PSUM"))
        ps_s = ctx.enter_context(tc.tile_pool(name="psS", bufs=2, space="PSUM"))
        ps_pv = ctx.enter_context(tc.tile_pool(name="psPV", bufs=2, space="PSUM"))

        # ---- local SBUF constants / persistent tiles ----
        warm = const.tile([128, QT], bf16, tag="warm")
        nc.gpsimd.memset(warm[:], 0.0)
        ident = const.tile([128, 128], bf16, tag="ident")
        masks.make_identity(nc, ident[:])
        # f32 selector column: picks the sumexp row (partition 64) via matmul
        sel = const.tile([65, 1], f32, tag="sel")
        nc.vector.memset(sel[:], 0.0)
        nc.vector.memset(sel[64:65, :], 1.0)
        # f32 ones row: partition-broadcast via PE (tail normalize)
        ones64 = const.tile([1, 64], f32, tag="ones64")
        nc.vector.memset(ones64[:], 1.0)

        wk_sb = const.tile([128, KC, DHC], bf16, tag="wk")
        bk_sb = const.tile([DHC, 1], f32, tag="bk")
        wv_sb = const.tile([128, KC, DHC], bf16, tag="wv")
        bv_sb = const.tile([DHC, 1], f32, tag="bv")
        wq_sb = const.tile([128, KC, DHC], bf16, tag="wq")
        bq_sb = const.tile([DHC, 1], f32, tag="bq")
        wo_sb = const.tile([128, D], bf16, tag="wo")
        # partition-0 copy of Wo's h1 rows for the tail's per-head split
        woL_sb = const.tile([64, D], bf16, tag="woL")

        qTs, kTs, vTs, vs, xTs = [], [], [], [], []
        for b in range(B):
            qTs.append(big.tile([128, N], bf16, tag=f"qT{b}", name=f"qT{b}"))
            kTs.append(big.tile([128, N], bf16, tag=f"kT{b}", name=f"kT{b}"))
            vTs.append(big.tile([128, N], bf16, tag=f"vT{b}", name=f"vT{b}"))
            v = big.tile([128, NKT, HPC, 65], bf16, tag=f"v{b}", name=f"v{b}")
            nc.gpsimd.memset(v[:, :, :, 64:65], 1.0)
            vs.append(v)
            xTs.append(opool.tile([128, N], bf16, tag=f"xT{b}", name=f"xT{b}"))

        # ---- PE warm-up: garbage matmuls lift the HAM clock-gate while the
        # first DMAs land (outputs never read) ----
        def warm_mms(n, pool, tag):
            for _ in range(n):
                psw = pool.tile([65, QT], f32, tag=tag, name="psw")
                nc.tensor.matmul(
                    psw[:], warm[:, 0:65], warm[:], start=True, stop=True
                )

        warm_mms(8, ps_proj, "proj")

        # ---- preload the exp activation table while ScalarE is idle ----
        junk = const.tile([128, 1], bf16, tag="junk")
        nc.scalar.activation(
            junk[:], warm[:, 0:1], mybir.ActivationFunctionType.Exp, scale=0.125
        )

        # ================= filler machinery =================
        fill_q = []
        passed = set()

        def pump(ns):
            spent = 0
            while fill_q and ns > spent:
                it = fill_q.pop(0)
                if isinstance(it, str):
                    passed.add(it)
                    continue
                spent += it()
            return spent

        def drain_to(tag):
            if tag in passed:
                return
            while fill_q:
                it = fill_q.pop(0)
                if isinstance(it, str):
                    passed.add(it)
                    if it == tag:
                        return
                    continue
                it()

        def flush():
            while fill_q:
                it = fill_q.pop(0)
                if isinstance(it, str):
                    passed.add(it)
                else:
                    it()

        def drain_free():
            # emit leading zero-cost ops (DMA issues) without touching matmuls
            while fill_q and (
                isinstance(fill_q[0], str) or getattr(fill_q[0], "free", False)
            ):
                it = fill_q.pop(0)
                if isinstance(it, str):
                    passed.add(it)
                else:
                    it()

        def run_gen(g):
            for f in g:
                f()

        # ---- projection generator: dst[dh, rows] = W^T x^T (+ bias) ----
        # yields micro-ops; V transposes trail their chunk by one chunk so the
        # PE never waits on the bias-add.
        def gen_proj(b, dst, xdram, w_sb, b_sb, xw, t0, t1, split=False,
                     vtile=None):
            pend = []
            for xi in range(t0, t1):
                rlo = b * N + xi * xw
                xt = xpool.tile([128, KC, 1024], bf16, tag="x", name="xt")
                src = _fold(xdram[:, rlo : rlo + xw])
                if split:
                    def d1(xt=xt, src=src, xw=xw):
                        nc.sync.dma_start(xt[:, 0:1, 0:xw], src[:, 0:1, :])
                        return 0
                    d1.free = True
                    yield d1
                    def d2(xt=xt, src=src, xw=xw):
                        nc.sync.dma_start(xt[:, 1:KC, 0:xw], src[:, 1:KC, :])
                        return 0
                    d2.free = True
                    yield d2
                else:
                    def d0(xt=xt, src=src, xw=xw):
                        nc.sync.dma_start(xt[:, :, 0:xw], src)
                        return 0
                    d0.free = True
                    yield d0
                for rl in range(xw // QT):
                    ps = ps_proj.tile([128, QT], f32, tag="proj", name="psp")
                    for kc in range(KC):
                        def mm(ps=ps, xt=xt, kc=kc, rl=rl):
                            nc.tensor.matmul(
                                ps[:],
                                w_sb[:, kc, :],
                                xt[:, kc, rl * QT : (rl + 1) * QT],
                                start=(kc == 0), stop=(kc == KC - 1),
                            )
                            return 215
                        yield mm
                    rt = (xi * xw + rl * QT) // QT
                    def badd(ps=ps, rt=rt):
                        nc.vector.tensor_scalar_add(
                            dst[:, rt * QT : (rt + 1) * QT], ps[:], b_sb[:]
                        )
                        return 0
                    yield badd
                    yield from pend
                    pend = []
                    if vtile is not None:
                        for kt in range(rt * 4, rt * 4 + 4):
                            pst = ps_proj.tile(
                                [128, 128], bf16, tag="proj", name="tp"
                            )
                            def tr(kt=kt, pst=pst):
                                nc.tensor.transpose(
                                    pst[:],
                                    dst[:, kt * 128 : (kt + 1) * 128],
                                    ident[:],
                                )
                                return 215
                            pend.append(tr)
                            def trc(kt=kt, pst=pst, vtile=vtile):
                                nc.vector.tensor_copy(
                                    vtile[:, kt, :, 0:64],
                                    pst[:].rearrange("p (h c) -> p h c", h=HPC),
                                )
                                return 0
                            pend.append(trc)
            yield from pend

        # ---- output projection generator for one (b, qt) ----
        def gen_outproj(b, qt):
            qs = slice(qt * QT, (qt + 1) * QT)
            for ot in range(KC):
                ps = ps_proj.tile([128, QT], f32, tag="proj", name="pso")
                def mm(ps=ps, ot=ot, b=b):
                    nc.tensor.matmul(
                        ps[:],
                        wo_sb[:, ot * 128 : (ot + 1) * 128],
                        xTs[b][:, qs],
                        start=True, stop=True,
                    )
                    return 215
                yield mm
                ob = opool.tile([128, QT], bf16, tag="o", name="ob")
                def cst(ps=ps, ob=ob):
                    nc.vector.tensor_copy(ob[:], ps[:])
                    return 0
                yield cst
                def st(ob=ob, ot=ot, b=b, qt=qt):
                    nc.sync.dma_start(
                        outT[
                            ot * 128 : (ot + 1) * 128,
                            b * N + qt * QT : b * N + (qt + 1) * QT,
                        ],
                        ob[:],
                    )
                    return 0
                yield st

        # ---- attention for one (b, qt): software-pipelined kt loop ----
        def attention_qt(b, qt, budget, kneed=None, vneed=None):
            drain_free()
            qs = slice(qt * QT, (qt + 1) * QT)
            pvs = [
                ps_pv.tile([65, QT], f32, tag="pv", name=f"pv{h}")
                for h in range(HPC)
            ]
            sgs, pts = {}, {}

            def scores(kt):
                if kneed and kt in kneed:
                    drain_to(kneed[kt])
                sg = ps_s.tile([128, 2 * QT], f32, tag="sg", name="sg")
                sgs[kt] = sg
                ks = slice(kt * KT, (kt + 1) * KT)
                for h in range(HPC):
                    hp = slice(64 * h, 64 * h + 64)
                    nc.tensor.matmul(
                        sg[:, h * QT : (h + 1) * QT],
                        kTs[b][hp, ks],
                        qTs[b][hp, qs],
                        start=True, stop=True,
                    )

            def expx(kt):
                pt = ppool.tile([128, 2 * QT], bf16, tag="pt", name="pt")
                pts[kt] = pt
                nc.scalar.activation(
                    pt[:], sgs.pop(kt)[:],
                    mybir.ActivationFunctionType.Exp, scale=0.125,
                )

            def pv(kt):
                if vneed and kt in vneed:
                    drain_to(vneed[kt])
                pt = pts.pop(kt)
                for h in range(HPC):
                    nc.tensor.matmul(
                        pvs[h][:],
                        vs[b][:, kt, h, :],
                        pt[:, h * QT : (h + 1) * QT],
                        start=(kt == 0), stop=(kt == NKT - 1),
                    )

            scores(0)
            expx(0)
            for kt in range(NKT):
                if kt + 1 < NKT:
                    scores(kt + 1)
                    expx(kt + 1)
                pump(budget)
                pv(kt)
            return pvs

        # ---- normalize X = PV / sumexp into xTs[b][:, qs] ----
        def normalize(b, qt, pvs):
            qs = slice(qt * QT, (qt + 1) * QT)
            for h in range(HPC):
                pvsb = npool.tile([65, QT], f32, tag="pvsb", name=f"pvsb{h}")
                nc.vector.tensor_copy(pvsb[:], pvs[h][:])
                rc = npool.tile([1, QT], f32, tag="rc", name=f"rc{h}")
                nc.sync.dma_start(rc[:], pvsb[64:65, :])
                nc.vector.reciprocal_approx_fast(rc[:], rc[:])
                rb = npool.tile([64, QT], f32, tag="rb", name=f"rb{h}")
                nc.gpsimd.partition_broadcast(rb[:], rc[:])
                if h == 0:
                    nc.vector.tensor_mul(xTs[b][0:64, qs], pvsb[0:64, :], rb[:])
                else:
                    tmp = npool.tile([64, QT], bf16, tag="tmp", name="tmp")
                    nc.vector.tensor_mul(tmp[:], pvsb[0:64, :], rb[:])
                    nc.sync.dma_start(xTs[b][64:128, qs], tmp[:])

        # ---- tail: last q-tile normalize + per-head-split output proj ----
        # gpsimd-free (its queue may sit behind store drains): the sumexp row
        # shift and the partition broadcast both run as PE matmuls.
        def tail_qt(b, qt, pvs):
            qs = slice(qt * QT, (qt + 1) * QT)
            xh, zr = [], []
            # copies first (h1 on ScalarE so the two heads run in parallel)
            pvsbs = []
            for h in range(HPC):
                pvsb = npool.tile([65, QT], f32, tag="pvsb", name=f"tpv{h}")
                if h == 0:
                    nc.vector.tensor_copy(pvsb[:], pvs[h][:])
                else:
                    nc.scalar.copy(pvsb[:], pvs[h][:])
                pvsbs.append(pvsb)
            # z-row shift via PE selector matmul (partition 64 -> 0)
            for h in range(HPC):
                zp = ps_pv.tile([1, QT], f32, tag="pv", name=f"zp{h}")
                nc.tensor.matmul(
                    zp[:], sel[:], pvsbs[h][:], start=True, stop=True
                )
                zr.append(zp)
            # junk matmuls keep the PE busy (HAM stays at full clock) while
            # the reciprocals run on DVE
            warm_mms(6, ps_s, "sg")
            rcs = []
            for h in range(HPC):
                rc = npool.tile([1, QT], f32, tag="rc", name=f"trc{h}")
                nc.vector.reciprocal_approx_fast(rc[:], zr[h][:])
                rcs.append(rc)
            # broadcast 1/z across 64 partitions via PE outer product
            rbs = []
            for h in range(HPC):
                rb = ps_pv.tile([64, QT], f32, tag="pv", name=f"trb{h}")
                nc.tensor.matmul(rb[:], ones64[:], rcs[h][:], start=True, stop=True)
                rbs.append(rb)
            warm_mms(6, ps_s, "sg")
            for h in range(HPC):
                xt = npool.tile([64, QT], bf16, tag="tmp", name=f"tx{h}")
                nc.vector.tensor_mul(xt[:], pvsbs[h][0:64, :], rbs[h][:])
                xh.append(xt)
            # output projection split by head (contraction halves accumulate)
            for ot in range(KC):
                pool, tag = (ps_proj, "proj") if ot % 2 == 0 else (ps_s, "sg")
                ps = pool.tile([128, QT], f32, tag=tag, name="tps")
                for h in range(HPC):
                    w = wo_sb if h == 0 else woL_sb
                    nc.tensor.matmul(
                        ps[:],
                        w[0:64, ot * 128 : (ot + 1) * 128],
                        xh[h][:],
                        start=(h == 0), stop=(h == 1),
                    )
                ob = opool.tile([128, QT], bf16, tag="o", name="tob")
                if ot % 2 == 0:
                    nc.vector.tensor_copy(ob[:], ps[:])
                else:
                    nc.scalar.copy(ob[:], ps[:])
                # split the final flush across both HWDGE rings
                eng = nc.sync if ot % 2 == 0 else nc.scalar
                eng.dma_start(
                    outT[
                        ot * 128 : (ot + 1) * 128,
                        b * N + qt * QT : b * N + (qt + 1) * QT,
                    ],
                    ob[:],
                )

        # ================= the schedule =================
        # Weight DMAs are issued in first-use order, interleaved with the
        # direct projection sections so the first x tile is never queued
        # behind weights it doesn't need yet.
        # K/V tiles 0,1 (xw=512 -> keys 0:1024) and Q tile 0 run directly so
        # attention(0, qt0) can start as soon as ~3MB of x has landed.
        nc.sync.dma_start(wk_sb[:], wk)
        nc.sync.dma_start(bk_sb[:], bk)
        run_gen(gen_proj(0, kTs[0], xk, wk_sb, bk_sb, 512, 0, 2, split=True))
        nc.sync.dma_start(wq_sb[:], wq)
        nc.sync.dma_start(bq_sb[:], bq)
        run_gen(gen_proj(0, qTs[0], xq, wq_sb, bq_sb, 512, 0, 1, split=True))
        nc.sync.dma_start(wv_sb[:], wv)
        nc.sync.dma_start(bv_sb[:], bv)
        nc.sync.dma_start(wo_sb[:], wo)
        nc.sync.dma_start(woL_sb[:], wo[64:128, :])

        for t in range(2):
            fill_q.extend(gen_proj(0, vTs[0], xv, wv_sb, bv_sb, 512, t, t + 1,
                                   split=(t == 0), vtile=vs[0]))
            fill_q.append(f"V0{'ab'[t]}")
        fill_q.extend(gen_proj(0, kTs[0], xk, wk_sb, bk_sb, 512, 2, 4))
        fill_q.append("K0")
        fill_q.extend(gen_proj(0, vTs[0], xv, wv_sb, bv_sb, 512, 2, 4,
                               vtile=vs[0]))
        fill_q.append("V0c")
        for t in range(1, 4):
            fill_q.extend(gen_proj(0, qTs[0], xq, wq_sb, bq_sb, 512, t, t + 1))
            fill_q.append(f"Q0_{t}")
        fill_q.extend(gen_proj(1, kTs[1], xk, wk_sb, bk_sb, 1024, 0, 2))
        fill_q.append("K1")
        fill_q.extend(gen_proj(1, vTs[1], xv, wv_sb, bv_sb, 1024, 0, 2,
                               vtile=vs[1]))
        fill_q.append("V1")
        for t in range(2):
            fill_q.extend(gen_proj(1, qTs[1], xq, wq_sb, bq_sb, 1024, t, t + 1))
            fill_q.append(f"Q1_{t}")

        for qt in range(NQT):
            if qt > 0:
                drain_to(f"Q0_{qt}")
            pvs = attention_qt(
                0, qt, budget=430,
                kneed={8: "K0"} if qt == 0 else None,
                vneed={0: "V0a", 4: "V0b", 8: "V0c"} if qt == 0 else None,
            )
            normalize(0, qt, pvs)
            fill_q.extend(gen_outproj(0, qt))

        for qt in range(NQT):
            # scores(qt) read qT[1][:, qs]: its projection fillers MUST be
            # emitted before the attention matmuls (in-order engine queues
            # cannot satisfy a dependency that sits behind the consumer)
            drain_to(f"Q1_{(qt * QT) // 1024}")
            pvs = attention_qt(
                1, qt, budget=500,
                kneed={0: "K1"} if qt == 0 else None,
                vneed={0: "V1"} if qt == 0 else None,
            )
            if qt < NQT - 1:
                normalize(1, qt, pvs)
                fill_q.extend(gen_outproj(1, qt))
            else:
                flush()
                tail_qt(1, qt, pvs)

    nc.compile()
    _cache["nc"] = nc
    return nc


def kernel(x_q, x_k, x_v, Wq, bq, Wk, bk, Wv, bv, Wo, bo, _trace=False):
    x_q = np.asarray(x_q, dtype=np.float32)
    x_k = np.asarray(x_k, dtype=np.float32)
    x_v = np.asarray(x_v, dtype=np.float32)
    Wq, Wk, Wv, Wo = (np.asarray(w, dtype=np.float32) for w in (Wq, Wk, Wv, Wo))
    bq, bk, bv, bo = (np.asarray(v, dtype=np.float32) for v in (bq, bk, bv, bo))

    bf = ml_dtypes.bfloat16
    xqT = np.ascontiguousarray(x_q.reshape(R, D).T).astype(bf)
    xkT = np.ascontiguousarray(x_k.reshape(R, D).T).astype(bf)
    xvT = np.ascontiguousarray(x_v.reshape(R, D).T).astype(bf)

    in_maps = []
    for c in range(NC):
        s = slice(DHC * c, DHC * (c + 1))
        in_maps.append(
            {
                "xqT": xqT,
                "xkT": xkT,
                "xvT": xvT,
                "wqT": _foldw(Wq[s, :].T).astype(bf),
                "wkT": _foldw(Wk[s, :].T).astype(bf),
                "wvT": _foldw(Wv[s, :].T).astype(bf),
                "woT": np.ascontiguousarray(Wo[:, s].T).astype(bf),
                "bq": bq[s][:, None].copy(),
                "bk": bk[s][:, None].copy(),
                "bv": bv[s][:, None].copy(),
            }
        )

    nc = build()
    res = run_bass_kernel_spmd(nc, in_maps, core_ids=list(range(NC)), trace=_trace)
    total = np.zeros((D, R), dtype=np.float32)
    for c in range(NC):
        total += res.results[c]["outT"].astype(np.float32)
    out = total.T + bo[None, :]
    if _trace:
        kernel.last_exec_time_ns = res.exec_time_ns
    return out.reshape(B, N, D).astype(np.float32)


# revision 37
# speedup vs baseline: 1.0753x; 1.0018x over previous
"""Multi-head attention (B=2, N=2048, D=1024, H=16) on 8 TRN2 NeuronCores.

Sharding: tensor-parallel over heads. Core c owns heads 2c, 2c+1 (a 128-wide
slice of the concat head dim). Each core:
  - projects Q^T, K^T, V^T (transposed layout [dh, rows]) for its heads over
    all B*N=4096 rows from host-transposed bf16 x^T inputs (W stationary,
    x moving -> all matmuls stream 512 moving cols)
  - V^T is PE-transposed (identity matmul) into natural [keys, dh] tiles with
    an appended ones-column per head (sumexp rides the PV matmul for free)
  - attention with transposed scores S^T[k, q] = K Q^T, exp on ScalarE
    (scale=1/8 folded in; no max-subtract needed: |scores/8| < ~4)
  - partial output projection out^T_c = Wo[:, slice] X_c^T -> [1024, 4096]
Host sums the 8 partial outputs and adds bo.

Schedule: one software-pipelined instruction stream. The attention kt loop is
scalar-bound (exp ~1.1us/kt vs ~0.86us of PE work), so all other PE work
(remaining projections, output projections) is queued as "filler" matmuls and
pumped between the scores and PV matmuls of each kt iteration; sentinels in
the filler queue let attention start as soon as the first row-tiles of K/V/Q
are projected (exp starts ~15us instead of ~50us). The PE stays continuously
busy, which also keeps the HAM clock-gate at full rate. PSUM: sg ring 2x2
banks + pv ring 2 banks + proj/out ring 2 banks = 8 banks exactly. The tail
(last q-tile) uses a PE-selector matmul instead of the slow SBUF-DMA shift
for the sumexp row, splits its output projection per head to skip the xT
assembly DMA, and runs junk warm-up matmuls through the normalize chain so
the final matmuls are not clock-gated.
"""

import sys

sys.path.insert(0, "/opt/trn_rl_repo")

from contextlib import ExitStack

import ml_dtypes
import numpy as np

import concourse.bass as bass
import concourse.mybir as mybir
import concourse.tile as tile
from concourse import bacc, masks
from concourse.bass_utils import run_bass_kernel_spmd

B, N, D, H, DH = 2, 2048, 1024, 16, 64
R = B * N  # 4096
NC = 8
HPC = H // NC  # 2 heads per core
DHC = HPC * DH  # 128 head dims per core
QT = 512  # query tile (psum bank of fp32)
KT = 128  # key tile (psum partitions)
NQT = N // QT  # 4
NKT = N // KT  # 16
KC = D // 128  # 8 contraction chunks

f32 = mybir.dt.float32
bf16 = mybir.dt.bfloat16

_cache = {}


def _fold(ap):
    # [D, X] dram -> [128, KC, X] partition-folded view for one-shot DMA
    return ap.rearrange("(a p) m -> p a m", p=128)


def _foldw(w):
    # [D, DHC] host weight -> [128, KC, DHC] partition-folded, contiguous
    return np.ascontiguousarray(w.reshape(KC, 128, DHC).transpose(1, 0, 2))


def build():
    if "nc" in _cache:
        return _cache["nc"]
    nc = bacc.Bacc("TRN2", target_bir_lowering=False, debug=False, num_devices=NC)
    xq = nc.dram_tensor("xqT", [D, R], bf16, kind="ExternalInput").ap()
    xk = nc.dram_tensor("xkT", [D, R], bf16, kind="ExternalInput").ap()
    xv = nc.dram_tensor("xvT", [D, R], bf16, kind="ExternalInput").ap()
    wq = nc.dram_tensor("wqT", [128, KC, DHC], bf16, kind="ExternalInput").ap()
    wk = nc.dram_tensor("wkT", [128, KC, DHC], bf16, kind="ExternalInput").ap()
    wv = nc.dram_tensor("wvT", [128, KC, DHC], bf16, kind="ExternalInput").ap()
    wo = nc.dram_tensor("woT", [DHC, D], bf16, kind="ExternalInput").ap()
    # bias columns packed [bk | bv | bq] so one DMA covers all three
    bqkv = nc.dram_tensor("bqkv", [DHC, 3], f32, kind="ExternalInput").ap()
    outT = nc.dram_tensor("outT", [D, R], bf16, kind="ExternalOutput").ap()

    with tile.TileContext(nc) as tc, ExitStack() as ctx:
        const = ctx.enter_context(tc.tile_pool(name="const", bufs=1))
        xpool = ctx.enter_context(tc.tile_pool(name="x", bufs=3))
        big = ctx.enter_context(tc.tile_pool(name="big", bufs=1))
        ppool = ctx.enter_context(tc.tile_pool(name="p", bufs=3))
        opool = ctx.enter_context(tc.tile_pool(name="o", bufs=4))
        npool = ctx.enter_context(tc.tile_pool(name="norm", bufs=2))
        ps_proj = ctx.enter_context(tc.tile_pool(name="psA", bufs=2, space="PSUM"))
        ps_s = ctx.enter_context(tc.tile_pool(name="psS", bufs=2, space="PSUM"))
        ps_pv = ctx.enter_context(tc.tile_pool(name="psPV", bufs=2, space="PSUM"))

        # ---- local SBUF constants / persistent tiles ----
        warm = const.tile([128, QT], bf16, tag="warm")
        nc.gpsimd.memset(warm[:], 0.0)
        ident = const.tile([128, 128], bf16, tag="ident")
        masks.make_identity(nc, ident[:])
        # f32 selector column: picks the sumexp row (partition 64) via matmul
        sel = const.tile([65, 1], f32, tag="sel")
        nc.vector.memset(sel[:], 0.0)
        nc.vector.memset(sel[64:65, :], 1.0)
        # f32 ones row: partition-broadcast via PE (tail normalize)
        ones64 = const.tile([1, 64], f32, tag="ones64")
        nc.vector.memset(ones64[:], 1.0)

        wk_sb = const.tile([128, KC, DHC], bf16, tag="wk")
        wv_sb = const.tile([128, KC, DHC], bf16, tag="wv")
        wq_sb = const.tile([128, KC, DHC], bf16, tag="wq")
        b_sb = const.tile([DHC, 3], f32, tag="bqkv")
        bk_sb, bv_sb, bq_sb = b_sb[:, 0:1], b_sb[:, 1:2], b_sb[:, 2:3]
        wo_sb = const.tile([128, D], bf16, tag="wo")
        # partition-0 copy of Wo's h1 rows for the tail's per-head split
        woL_sb = const.tile([64, D], bf16, tag="woL")

        qTs, kTs, vTs, vs, xTs = [], [], [], [], []
        for b in range(B):
            qTs.append(big.tile([128, N], bf16, tag=f"qT{b}", name=f"qT{b}"))
            kTs.append(big.tile([128, N], bf16, tag=f"kT{b}", name=f"kT{b}"))
            vTs.append(big.tile([128, N], bf16, tag=f"vT{b}", name=f"vT{b}"))
            v = big.tile([128, NKT, HPC, 65], bf16, tag=f"v{b}", name=f"v{b}")
            nc.gpsimd.memset(v[:, :, :, 64:65], 1.0)
            vs.append(v)
            xTs.append(opool.tile([128, N], bf16, tag=f"xT{b}", name=f"xT{b}"))

        # ---- PE warm-up: garbage matmuls lift the HAM clock-gate while the
        # first DMAs land (outputs never read) ----
        def warm_mms(n, pool, tag):
            for _ in range(n):
                psw = pool.tile([65, QT], f32, tag=tag, name="psw")
                nc.tensor.matmul(
                    psw[:], warm[:, 0:65], warm[:], start=True, stop=True
                )

        warm_mms(8, ps_proj, "proj")

        # ---- preload the exp activation table while ScalarE is idle ----
        junk = const.tile([128, 1], bf16, tag="junk")
        nc.scalar.activation(
            junk[:], warm[:, 0:1], mybir.ActivationFunctionType.Exp, scale=0.125
        )

        # ================= filler machinery =================
        fill_q = []
        passed = set()

        def pump(ns):
            spent = 0
            while fill_q and ns > spent:
                it = fill_q.pop(0)
                if isinstance(it, str):
                    passed.add(it)
                    continue
                spent += it()
            return spent

        def drain_to(tag):
            if tag in passed:
                return
            while fill_q:
                it = fill_q.pop(0)
                if isinstance(it, str):
                    passed.add(it)
                    if it == tag:
                        return
                    continue
                it()

        def flush():
            while fill_q:
                it = fill_q.pop(0)
                if isinstance(it, str):
                    passed.add(it)
                else:
                    it()

        def drain_free():
            # emit leading zero-cost ops (DMA issues) without touching matmuls
            while fill_q and (
                isinstance(fill_q[0], str) or getattr(fill_q[0], "free", False)
            ):
                it = fill_q.pop(0)
                if isinstance(it, str):
                    passed.add(it)
                else:
                    it()

        def run_gen(g):
            for f in g:
                f()

        # ---- projection generator: dst[dh, rows] = W^T x^T (+ bias) ----
        # yields micro-ops; V transposes trail their chunk by one chunk so the
        # PE never waits on the bias-add.
        def gen_proj(b, dst, xdram, w_sb, b_sb, xw, t0, t1, split=False,
                     vtile=None):
            pend = []
            for xi in range(t0, t1):
                rlo = b * N + xi * xw
                xt = xpool.tile([128, KC, 1024], bf16, tag="x", name="xt")
                src = _fold(xdram[:, rlo : rlo + xw])
                if split:
                    def d1(xt=xt, src=src, xw=xw):
                        nc.sync.dma_start(xt[:, 0:1, 0:xw], src[:, 0:1, :])
                        return 0
                    d1.free = True
                    yield d1
                    def d2(xt=xt, src=src, xw=xw):
                        nc.sync.dma_start(xt[:, 1:KC, 0:xw], src[:, 1:KC, :])
                        return 0
                    d2.free = True
                    yield d2
                else:
                    def d0(xt=xt, src=src, xw=xw):
                        nc.sync.dma_start(xt[:, :, 0:xw], src)
                        return 0
                    d0.free = True
                    yield d0
                for rl in range(xw // QT):
                    ps = ps_proj.tile([128, QT], f32, tag="proj", name="psp")
                    for kc in range(KC):
                        def mm(ps=ps, xt=xt, kc=kc, rl=rl):
                            nc.tensor.matmul(
                                ps[:],
                                w_sb[:, kc, :],
                                xt[:, kc, rl * QT : (rl + 1) * QT],
                                start=(kc == 0), stop=(kc == KC - 1),
                            )
                            return 215
                        yield mm
                    rt = (xi * xw + rl * QT) // QT
                    def badd(ps=ps, rt=rt):
                        nc.vector.tensor_scalar_add(
                            dst[:, rt * QT : (rt + 1) * QT], ps[:], b_sb[:]
                        )
                        return 0
                    yield badd
                    yield from pend
                    pend = []
                    if vtile is not None:
                        for kt in range(rt * 4, rt * 4 + 4):
                            pst = ps_proj.tile(
                                [128, 128], bf16, tag="proj", name="tp"
                            )
                            def tr(kt=kt, pst=pst):
                                nc.tensor.transpose(
                                    pst[:],
                                    dst[:, kt * 128 : (kt + 1) * 128],
                                    ident[:],
                                )
                                return 215
                            pend.append(tr)
                            def trc(kt=kt, pst=pst, vtile=vtile):
                                nc.vector.tensor_copy(
                                    vtile[:, kt, :, 0:64],
                                    pst[:].rearrange("p (h c) -> p h c", h=HPC),
                                )
                                return 0
                            pend.append(trc)
            yield from pend

        # ---- output projection generator for one (b, qt) ----
        def gen_outproj(b, qt):
            qs = slice(qt * QT, (qt + 1) * QT)
            for ot in range(KC):
                ps = ps_proj.tile([128, QT], f32, tag="proj", name="pso")
                def mm(ps=ps, ot=ot, b=b):
                    nc.tensor.matmul(
                        ps[:],
                        wo_sb[:, ot * 128 : (ot + 1) * 128],
                        xTs[b][:, qs],
                        start=True, stop=True,
                    )
                    return 215
                yield mm
                ob = opool.tile([128, QT], bf16, tag="o", name="ob")
                def cst(ps=ps, ob=ob):
                    nc.vector.tensor_copy(ob[:], ps[:])
                    return 0
                yield cst
                def st(ob=ob, ot=ot, b=b, qt=qt):
                    nc.sync.dma_start(
                        outT[
                            ot * 128 : (ot + 1) * 128,
                            b * N + qt * QT : b * N + (qt + 1) * QT,
                        ],
                        ob[:],
                    )
                    return 0
                yield st

        # ---- attention for one (b, qt): software-pipelined kt loop ----
        def attention_qt(b, qt, budget, kneed=None, vneed=None):
            drain_free()
            qs = slice(qt * QT, (qt + 1) * QT)
            pvs = [
                ps_pv.tile([65, QT], f32, tag="pv", name=f"pv{h}")
                for h in range(HPC)
            ]
            sgs, pts = {}, {}

            def scores(kt):
                if kneed and kt in kneed:
                    drain_to(kneed[kt])
                sg = ps_s.tile([128, 2 * QT], f32, tag="sg", name="sg")
                sgs[kt] = sg
                ks = slice(kt * KT, (kt + 1) * KT)
                for h in range(HPC):
                    hp = slice(64 * h, 64 * h + 64)
                    nc.tensor.matmul(
                        sg[:, h * QT : (h + 1) * QT],
                        kTs[b][hp, ks],
                        qTs[b][hp, qs],
                        start=True, stop=True,
                    )

            def expx(kt):
                pt = ppool.tile([128, 2 * QT], bf16, tag="pt", name="pt")
                pts[kt] = pt
                nc.scalar.activation(
                    pt[:], sgs.pop(kt)[:],
                    mybir.ActivationFunctionType.Exp, scale=0.125,
                )

            def pv(kt):
                if vneed and kt in vneed:
                    drain_to(vneed[kt])
                pt = pts.pop(kt)
                for h in range(HPC):
                    nc.tensor.matmul(
                        pvs[h][:],
                        vs[b][:, kt, h, :],
                        pt[:, h * QT : (h + 1) * QT],
                        start=(kt == 0), stop=(kt == NKT - 1),
                    )

            scores(0)
            expx(0)
            for kt in range(NKT):
                if kt + 1 < NKT:
                    scores(kt + 1)
                    expx(kt + 1)
                pump(budget)
                pv(kt)
            return pvs

        # ---- normalize X = PV / sumexp into xTs[b][:, qs] ----
        def normalize(b, qt, pvs):
            qs = slice(qt * QT, (qt + 1) * QT)
            for h in range(HPC):
                pvsb = npool.tile([65, QT], f32, tag="pvsb", name=f"pvsb{h}")
                nc.vector.tensor_copy(pvsb[:], pvs[h][:])
                rc = npool.tile([1, QT], f32, tag="rc", name=f"rc{h}")
                nc.sync.dma_start(rc[:], pvsb[64:65, :])
                nc.vector.reciprocal_approx_fast(rc[:], rc[:])
                rb = npool.tile([64, QT], f32, tag="rb", name=f"rb{h}")
                nc.gpsimd.partition_broadcast(rb[:], rc[:])
                if h == 0:
                    nc.vector.tensor_mul(xTs[b][0:64, qs], pvsb[0:64, :], rb[:])
                else:
                    tmp = npool.tile([64, QT], bf16, tag="tmp", name="tmp")
                    nc.vector.tensor_mul(tmp[:], pvsb[0:64, :], rb[:])
                    nc.sync.dma_start(xTs[b][64:128, qs], tmp[:])

        # ---- tail: last q-tile normalize + per-head-split output proj ----
        # gpsimd-free (its queue may sit behind store drains): the sumexp row
        # shift and the partition broadcast both run as PE matmuls.
        def tail_qt(b, qt, pvs):
            qs = slice(qt * QT, (qt + 1) * QT)
            xh, zr = [], []
            # copies first (h1 on ScalarE so the two heads run in parallel)
            pvsbs = []
            for h in range(HPC):
                pvsb = npool.tile([65, QT], f32, tag="pvsb", name=f"tpv{h}")
                if h == 0:
                    nc.vector.tensor_copy(pvsb[:], pvs[h][:])
                else:
                    nc.scalar.copy(pvsb[:], pvs[h][:])
                pvsbs.append(pvsb)
            # z-row shift via PE selector matmul (partition 64 -> 0)
            for h in range(HPC):
                zp = ps_pv.tile([1, QT], f32, tag="pv", name=f"zp{h}")
                nc.tensor.matmul(
                    zp[:], sel[:], pvsbs[h][:], start=True, stop=True
                )
                zr.append(zp)
            # junk matmuls keep the PE busy (HAM stays at full clock) while
            # the reciprocals run on DVE
            warm_mms(6, ps_s, "sg")
            rcs = []
            for h in range(HPC):
                rc = npool.tile([1, QT], f32, tag="rc", name=f"trc{h}")
                nc.vector.reciprocal_approx_fast(rc[:], zr[h][:])
                rcs.append(rc)
            # broadcast 1/z across 64 partitions via PE outer product
            rbs = []
            for h in range(HPC):
                rb = ps_pv.tile([64, QT], f32, tag="pv", name=f"trb{h}")
                nc.tensor.matmul(rb[:], ones64[:], rcs[h][:], start=True, stop=True)
                rbs.append(rb)
            warm_mms(6, ps_s, "sg")
            for h in range(HPC):
                xt = npool.tile([64, QT], bf16, tag="tmp", name=f"tx{h}")
                nc.vector.tensor_mul(xt[:], pvsbs[h][0:64, :], rbs[h][:])
                xh.append(xt)
            # output projection split by head (contraction halves accumulate)
            for ot in range(KC):
                pool, tag = (ps_proj, "proj") if ot % 2 == 0 else (ps_s, "sg")
                ps = pool.tile([128, QT], f32, tag=tag, name="tps")
                for h in range(HPC):
                    w = wo_sb if h == 0 else woL_sb
                    nc.tensor.matmul(
                        ps[:],
                        w[0:64, ot * 128 : (ot + 1) * 128],
                        xh[h][:],
                        start=(h == 0), stop=(h == 1),
                    )
                ob = opool.tile([128, QT], bf16, tag="o", name="tob")
                if ot % 2 == 0:
                    nc.vector.tensor_copy(ob[:], ps[:])
                else:
                    nc.scalar.copy(ob[:], ps[:])
                # split the final flush across both HWDGE rings
                eng = nc.sync if ot % 2 == 0 else nc.scalar
                eng.dma_start(
                    outT[
                        ot * 128 : (ot + 1) * 128,
                        b * N + qt * QT : b * N + (qt + 1) * QT,
                    ],
                    ob[:],
                )

        # ================= the schedule =================
        # Weight DMAs are issued in first-use order, interleaved with the
        # direct projection sections so the first x tile is never queued
        # behind weights it doesn't need yet.
        # K/V tiles 0,1 (xw=512 -> keys 0:1024) and Q tile 0 run directly so
        # attention(0, qt0) can start as soon as ~3MB of x has landed.
        nc.sync.dma_start(wk_sb[:], wk)
        nc.sync.dma_start(b_sb[:], bqkv)
        run_gen(gen_proj(0, kTs[0], xk, wk_sb, bk_sb, 512, 0, 1, split=True))
        run_gen(gen_proj(0, kTs[0], xk, wk_sb, bk_sb, 512, 1, 2))
        nc.sync.dma_start(wq_sb[:], wq)
        run_gen(gen_proj(0, qTs[0], xq, wq_sb, bq_sb, 512, 0, 1))
        nc.sync.dma_start(wv_sb[:], wv)

        def wo_dma():
            nc.sync.dma_start(wo_sb[:], wo)
            nc.sync.dma_start(woL_sb[:], wo[64:128, :])
            return 0
        wo_dma.free = True

        for t in range(2):
            fill_q.extend(gen_proj(0, vTs[0], xv, wv_sb, bv_sb, 512, t, t + 1,
                                   vtile=vs[0]))
            fill_q.append(f"V0{'ab'[t]}")
        fill_q.append(wo_dma)
        fill_q.extend(gen_proj(0, kTs[0], xk, wk_sb, bk_sb, 512, 2, 4))
        fill_q.append("K0")
        fill_q.extend(gen_proj(0, vTs[0], xv, wv_sb, bv_sb, 512, 2, 4,
                               vtile=vs[0]))
        fill_q.append("V0c")
        for t in range(1, 4):
            fill_q.extend(gen_proj(0, qTs[0], xq, wq_sb, bq_sb, 512, t, t + 1))
            fill_q.append(f"Q0_{t}")
        fill_q.extend(gen_proj(1, kTs[1], xk, wk_sb, bk_sb, 1024, 0, 2))
        fill_q.append("K1")
        fill_q.extend(gen_proj(1, vTs[1], xv, wv_sb, bv_sb, 1024, 0, 2,
                               vtile=vs[1]))
        fill_q.append("V1")
        for t in range(2):
            fill_q.extend(gen_proj(1, qTs[1], xq, wq_sb, bq_sb, 1024, t, t + 1))
            fill_q.append(f"Q1_{t}")

        for qt in range(NQT):
            if qt > 0:
                drain_to(f"Q0_{qt}")
            pvs = attention_qt(
                0, qt, budget=430,
                kneed={8: "K0"} if qt == 0 else None,
                vneed={0: "V0a", 4: "V0b", 8: "V0c"} if qt == 0 else None,
            )
            normalize(0, qt, pvs)
            fill_q.extend(gen_outproj(0, qt))

        for qt in range(NQT):
            # scores(qt) read qT[1][:, qs]: its projection fillers MUST be
            # emitted before the attention matmuls (in-order engine queues
            # cannot satisfy a dependency that sits behind the consumer)
            drain_to(f"Q1_{(qt * QT) // 1024}")
            pvs = attention_qt(
                1, qt, budget=500,
                kneed={0: "K1"} if qt == 0 else None,
                vneed={0: "V1"} if qt == 0 else None,
            )
            if qt < NQT - 1:
                normalize(1, qt, pvs)
                fill_q.extend(gen_outproj(1, qt))
            else:
                flush()
                tail_qt(1, qt, pvs)

    nc.compile()
    _cache["nc"] = nc
    return nc


def kernel(x_q, x_k, x_v, Wq, bq, Wk, bk, Wv, bv, Wo, bo, _trace=False):
    x_q = np.asarray(x_q, dtype=np.float32)
    x_k = np.asarray(x_k, dtype=np.float32)
    x_v = np.asarray(x_v, dtype=np.float32)
    Wq, Wk, Wv, Wo = (np.asarray(w, dtype=np.float32) for w in (Wq, Wk, Wv, Wo))
    bq, bk, bv, bo = (np.asarray(v, dtype=np.float32) for v in (bq, bk, bv, bo))

    bf = ml_dtypes.bfloat16
    xqT = np.ascontiguousarray(x_q.reshape(R, D).T).astype(bf)
    xkT = np.ascontiguousarray(x_k.reshape(R, D).T).astype(bf)
    xvT = np.ascontiguousarray(x_v.reshape(R, D).T).astype(bf)

    in_maps = []
    for c in range(NC):
        s = slice(DHC * c, DHC * (c + 1))
        in_maps.append(
            {
                "xqT": xqT,
                "xkT": xkT,
                "xvT": xvT,
                "wqT": _foldw(Wq[s, :].T).astype(bf),
                "wkT": _foldw(Wk[s, :].T).astype(bf),
                "wvT": _foldw(Wv[s, :].T).astype(bf),
                "woT": np.ascontiguousarray(Wo[:, s].T).astype(bf),
                "bqkv": np.stack([bk[s], bv[s], bq[s]], axis=1).astype(np.float32),
            }
        )

    nc = build()
    res = run_bass_kernel_spmd(nc, in_maps, core_ids=list(range(NC)), trace=_trace)
    total = np.zeros((D, R), dtype=np.float32)
    for c in range(NC):
        total += res.results[c]["outT"].astype(np.float32)
    out = total.T + bo[None, :]
    if _trace:
        kernel.last_exec_time_ns = res.exec_time_ns
    return out.reshape(B, N, D).astype(np.float32)
